# revision 24
# baseline (speedup 1.0000x reference)
"""Multi-head attention (B=2, N=2048, D=1024, H=16) on 8 trn2 NeuronCores.

Sharding: tensor-parallel over heads — core c owns heads (2c, 2c+1) for both
batches.  W_qkv columns / W_out rows are sliced per core on the host; each
core computes a full-size partial output (its heads' contribution through
W_out), and the host sums the 8 partials ("unshard" of the row-sharded W_out
product) and transposes back.

Device-side dataflow per core (all matmuls bf16, PSUM fp32):
  phase 1: QT = Wq^T x^T, KT = Wk^T x^T   ([dk*2, R] with dk on partitions)
           V  = x Wv                       ([R, dk*2] natural, + ones column)
  phase 2: scoresT_h = (KT_h)^T-matmul → [Nk, Nq] tiles; one exp per k-tile
           covering both heads (ACT, scale=1/sqrt(dk)); ctxT_h & softmax
           denominators from one M=65 matmul chain (lhsT = [V_h | m]); the
           0/1 attention mask is folded multiplicatively into V rows and the
           denominator column (exact); normalize via DVE reciprocal +
           gpsimd partition_broadcast.
  phase 3: outT_partial = Wo_c^T ctxT (+ b_out on core 0 only).
"""

import os
import sys
import functools

import numpy as np
import ml_dtypes

for _p in (
    "/root/.axon_site",
    "/root/.axon_site/_ro/trn_rl_repo",
    "/root/.axon_site/_ro/pypackages",
    "/opt/trn_rl_repo",
):
    if os.path.isdir(_p) and _p not in sys.path:
        sys.path.append(_p)

def _ensure_axon():
    """Register the axon PJRT backend if sitecustomize didn't run
    (e.g. kernel.py imported without the image's PYTHONPATH)."""
    import jax
    try:
        backends = jax._src.xla_bridge._backend_factories
        if "axon" in backends:
            return
    except Exception:
        pass
    try:
        from trn_agent_boot.trn_boot import boot
        os.environ.setdefault("AXON_POOL_SVC_OVERRIDE", "127.0.0.1")
        os.environ.setdefault("AXON_LOOPBACK_RELAY", "1")
        boot(os.environ["TRN_TERMINAL_PRECOMPUTED_JSON"],
             "/opt/axon/libaxon_pjrt.so")
    except Exception:
        pass


_ensure_axon()

import concourse.tile as tile
from concourse import bacc, mybir
from concourse.bass_utils import run_bass_kernel_spmd

BF16 = mybir.dt.bfloat16
F32 = mybir.dt.float32
I32 = mybir.dt.int32

B, N, D, H = 2, 2048, 1024, 16
DK = D // H            # 64
CORES = 8
HPC = H // CORES       # 2 heads per core
R = B * N              # 4096 rows total
RB = N                 # rows per batch
KT8 = D // 128         # 8 k-tiles over the model dim
CH = R // 512          # 8 row chunks of 512
QC = RB // 512         # 4 query chunks of 512 per batch
NKT = RB // 128        # 16 key tiles of 128 per batch
OT = D // 128          # 8 output-dim tiles


def _build_nc():
    nc = bacc.Bacc(
        "TRN2", target_bir_lowering=False, debug=False, enable_asserts=False,
        num_devices=CORES,
    )

    xT_d = nc.dram_tensor("xT", [D, R], BF16, kind="ExternalInput").ap()
    wq_d = nc.dram_tensor("wq", [128, KT8, 128], BF16, kind="ExternalInput").ap()
    wk_d = nc.dram_tensor("wk", [128, KT8, 128], BF16, kind="ExternalInput").ap()
    wv_d = nc.dram_tensor("wv", [128, KT8, 128], BF16, kind="ExternalInput").ap()
    wo_d = nc.dram_tensor("wo", [128, D], BF16, kind="ExternalInput").ap()
    bqkv_d = nc.dram_tensor("bqkv", [3, 128], F32, kind="ExternalInput").ap()
    maskT_d = nc.dram_tensor("maskT", [128, B * NKT], I32, kind="ExternalInput").ap()
    outT_d = nc.dram_tensor("outT", [D, R], F32, kind="ExternalOutput").ap()

    with tile.TileContext(nc) as tc:
        with (
            tc.tile_pool(name="persist", bufs=1) as persist,
            tc.tile_pool(name="xt", bufs=3) as xt_pool,
            tc.tile_pool(name="attn", bufs=5) as attn_pool,
            tc.tile_pool(name="small", bufs=4) as small,
            tc.tile_pool(name="outsb", bufs=2) as outsb_pool,
            tc.tile_pool(name="ps", bufs=2, space="PSUM") as ps_pool,
        ):
            # ---- constants / weights to SBUF ----
            wq_sb = persist.tile([128, KT8, 128], BF16, tag="wq")
            wk_sb = persist.tile([128, KT8, 128], BF16, tag="wk")
            wv_sb = persist.tile([128, KT8, 128], BF16, tag="wv")
            for w_sb, w_d in ((wq_sb, wq_d), (wk_sb, wk_d), (wv_sb, wv_d)):
                nc.sync.dma_start(out=w_sb, in_=w_d)
            wo_sb = persist.tile([128, D], BF16, tag="wo")

            bq_sb = persist.tile([128, 1], F32, tag="bq")
            bk_sb = persist.tile([128, 1], F32, tag="bk")
            bv_rep = persist.tile([128, 128], F32, tag="bv")
            maski = persist.tile([128, B * NKT], I32, tag="maski")
            maskf = persist.tile([128, B * NKT], F32, tag="maskf")

            def load_small_consts():
                nc.sync.dma_start(
                    out=bq_sb, in_=bqkv_d[0:1, :].rearrange("o p -> p o"))
                nc.sync.dma_start(
                    out=bk_sb, in_=bqkv_d[1:2, :].rearrange("o p -> p o"))
                nc.sync.dma_start(
                    out=bv_rep, in_=bqkv_d[2:3, :].partition_broadcast(128))
                nc.sync.dma_start(out=maski, in_=maskT_d)
                nc.vector.tensor_copy(out=maskf, in_=maski)


            # ---- persistent activations (split per batch for scheduling) ----
            qt_sb = [persist.tile([128, RB], BF16, tag=f"qt{b}", name=f"qt{b}") for b in range(B)]
            kt_sb = [persist.tile([128, RB], BF16, tag=f"kt{b}", name=f"kt{b}") for b in range(B)]
            # V rows with a ones column appended per head: [.., 64] = V, [64] = 1
            v_sb = [persist.tile([128, NKT, HPC, 66], BF16, tag=f"v{b}", name=f"v{b}")
                    for b in range(B)]

            ctxn_sb = [persist.tile([128, RB], BF16, tag=f"ctxn{b}", name=f"ctxn{b}")
                       for b in range(B)]

            # ---- phase 1: projections ----
            pending_v = {}

            def emit_v(b, chb, xt):
                    for sub in range(4):
                        rt = chb * 4 + sub
                        vps = ps_pool.tile([128, 128], F32, tag="mm1", name="vps")
                        for kt in range(KT8):
                            nc.tensor.matmul(
                                vps,
                                lhsT=xt[:, kt, sub * 128:(sub + 1) * 128],
                                rhs=wv_sb[:, kt, :],
                                start=(kt == 0), stop=(kt == KT8 - 1),
                            )
                        mcol = maskf[:, b * NKT + rt:b * NKT + rt + 1]
                        for h in range(HPC):
                            vslice = v_sb[b][:, rt, h, 0:64]
                            nc.vector.tensor_add(
                                out=vslice,
                                in0=vps[:, h * 64:(h + 1) * 64],
                                in1=bv_rep[:, h * 64:(h + 1) * 64],
                            )
                            nc.vector.tensor_scalar_mul(
                                out=vslice, in0=vslice, scalar1=mcol,
                            )
                            nc.vector.tensor_copy(
                                out=v_sb[b][:, rt, h, 64:65], in_=mcol,
                            )

            def phase1(b, chunks=None, after_dma=None):
                for chb in (chunks if chunks is not None else range(CH // B)):
                    roff = b * RB + chb * 512
                    xt = xt_pool.tile([128, KT8, 512], BF16, tag="xt")
                    for kt in range(KT8):
                        nc.sync.dma_start(
                            out=xt[:, kt, :],
                            in_=xT_d[kt * 128:(kt + 1) * 128, roff:roff + 512],
                        )
                    if after_dma is not None:
                        after_dma()
                        after_dma = None
                    for w_sb, b_sb, dest in (
                        (wq_sb, bq_sb, qt_sb[b]),
                        (wk_sb, bk_sb, kt_sb[b]),
                    ):
                        ps = ps_pool.tile([128, 512], F32, tag="mm1", name="qkvps")
                        for kt in range(KT8):
                            nc.tensor.matmul(
                                ps, lhsT=w_sb[:, kt, :], rhs=xt[:, kt, :],
                                start=(kt == 0), stop=(kt == KT8 - 1),
                            )
                        nc.vector.tensor_scalar_add(
                            out=dest[:, chb * 512:(chb + 1) * 512],
                            in0=ps, scalar1=b_sb,
                        )
                    if b in pending_v:
                        emit_v(b, *pending_v.pop(b))
                    pending_v[b] = (chb, xt)

            def phase1_flush(b):
                if b in pending_v:
                    emit_v(b, *pending_v.pop(b))

            # ---- phase 2: attention for batch b ----
            def attn_alloc():
                aw = [attn_pool.tile([128, NKT // 2, HPC, 512], BF16, tag="aw",
                                     name="awh") for _ in range(2)]
                cps = [ps_pool.tile([65, 512], F32, tag="ctxps", name=f"ctxps{h}")
                       for h in range(HPC)]
                return aw, cps

            def attn_scores(b, qc, aw, kt_lo, kt_hi, cps=None, interleave=False):
                qs = qc * 512
                for kt in range(kt_lo, kt_hi):
                    ks = kt * 128
                    sc = ps_pool.tile([128, HPC, 512], F32, tag="scps",
                                      name="scps")
                    for h in range(HPC):
                        nc.tensor.matmul(
                            sc[:, h, :],
                            lhsT=kt_sb[b][h * 64:(h + 1) * 64, ks:ks + 128],
                            rhs=qt_sb[b][h * 64:(h + 1) * 64, qs:qs + 512],
                            start=True, stop=True,
                            tile_position=(h * 64, 0),
                        )
                    nc.scalar.activation(
                        out=aw[kt // (NKT // 2)][:, kt % (NKT // 2), :, :],
                        in_=sc,
                        func=mybir.ActivationFunctionType.Exp,
                        scale=DK ** -0.5,
                    )
                    if interleave:
                        for h in range(HPC):
                            nc.tensor.matmul(
                                cps[h],
                                lhsT=v_sb[b][:, kt, h, 0:65],
                                rhs=aw[kt // (NKT // 2)][:, kt % (NKT // 2), h, :],
                                start=(kt == 0), stop=(kt == NKT - 1),
                                skip_group_check=True,
                            )

            def attn_ctx(b, aw, cps):
                for kt in range(NKT):
                    for h in range(HPC):
                        nc.tensor.matmul(
                            cps[h],
                            lhsT=v_sb[b][:, kt, h, 0:65],
                            rhs=aw[kt // (NKT // 2)][:, kt % (NKT // 2), h, :],
                            start=(kt == 0), stop=(kt == NKT - 1),
                        )

            def attn_norm(b, qc, cps):
                qs = qc * 512
                ctxfs = []
                for h in range(HPC):
                    ctxf = small.tile([65, 512], F32, tag="ctxf", name="ctxf")
                    nc.vector.tensor_copy(out=ctxf, in_=cps[h])
                    ctxfs.append(ctxf)
                for h in range(HPC):
                    ctxf = ctxfs[h]
                    rcp = small.tile([1, 512], F32, tag="rcp", name="rcp")
                    nc.vector.reciprocal(rcp, ctxf[64:65, :])
                    rep = small.tile([64, 512], F32, tag="rep", name="rep")
                    nc.gpsimd.partition_broadcast(rep, rcp)
                    nc.vector.tensor_mul(
                        out=ctxn_sb[b][h * 64:(h + 1) * 64, qs:qs + 512],
                        in0=ctxf[0:64, :], in1=rep,
                    )

            def phase2(b, qcs, after_chunk=None, mid_hooks=None):
                for qc in qcs:
                    last_chunk = (b == 1 and qc == QC - 1)
                    aw, cps = attn_alloc()
                    lo = 0
                    if last_chunk and mid_hooks:
                        for at_kt in sorted(mid_hooks):
                            attn_scores(b, qc, aw, lo, at_kt, cps=cps,
                                        interleave=True)
                            mid_hooks[at_kt]()
                            lo = at_kt
                    attn_scores(b, qc, aw, lo, NKT, cps=cps,
                                interleave=last_chunk)
                    if not last_chunk:
                        attn_ctx(b, aw, cps)
                    attn_norm(b, qc, cps)
                    if after_chunk is not None:
                        after_chunk(qc)

            # ---- phase 3: output projection (partial, this core's heads) ----
            def phase3(b, rc):
                osb = outsb_pool.tile([128, OT, 512], F32, tag="osb")
                tail = (b == 1 and rc == QC - 1)
                for ot in range(OT):
                    ps = ps_pool.tile([128, 512], F32, tag="mm1", name="outps")
                    nc.tensor.matmul(
                        ps,
                        lhsT=wo_sb[:, ot * 128:(ot + 1) * 128],
                        rhs=ctxn_sb[b][:, rc * 512:(rc + 1) * 512],
                        start=True, stop=True,
                    )
                    if tail and ot % 2 == 1:
                        nc.scalar.copy(out=osb[:, ot, :], in_=ps)
                    else:
                        nc.vector.tensor_copy(out=osb[:, ot, :], in_=ps)
                cs = b * RB + rc * 512
                outT_r = outT_d.rearrange("(t p) r -> p t r", p=128)
                step = 1 if tail else 2
                for j in range(0, OT, step):
                    nc.sync.dma_start(
                        out=outT_r[:, j:j + step, cs:cs + 512],
                        in_=osb[:, j:j + step, :],
                    )

            load_small_consts()
            phase1(0, chunks=[0])
            aw00, cps00 = attn_alloc()
            attn_scores(0, 0, aw00, 0, 4)
            phase1(0, chunks=[1])
            attn_scores(0, 0, aw00, 4, 8)
            phase1(0, chunks=[2])
            attn_scores(0, 0, aw00, 8, 12)
            phase1(0, chunks=[3])
            attn_scores(0, 0, aw00, 12, NKT)
            phase1_flush(0)
            attn_ctx(0, aw00, cps00)
            attn_norm(0, 0, cps00)
            phase2(0, range(1, QC))
            nc.sync.dma_start(out=wo_sb, in_=wo_d)
            phase1(1, chunks=[0])
            aw10, cps10 = attn_alloc()
            attn_scores(1, 0, aw10, 0, 4)
            phase1(1, chunks=[1])
            attn_scores(1, 0, aw10, 4, 8)
            phase1(1, chunks=[2])
            attn_scores(1, 0, aw10, 8, 12)
            phase1(1, chunks=[3])
            attn_scores(1, 0, aw10, 12, NKT)
            phase1_flush(1)
            attn_ctx(1, aw10, cps10)
            attn_norm(1, 0, cps10)
            phase3(0, 0)
            phase3(0, 1)
            phase3(1, 0)

            def tail_chunks(qc):
                if qc == 1:
                    phase3(0, 2)
                if qc != 2:
                    phase3(1, qc)

            phase2(1, range(1, QC), after_chunk=tail_chunks,
                   mid_hooks={8: lambda: phase3(1, 2),
                              12: lambda: phase3(0, 3)})

    nc.compile()
    return nc


@functools.lru_cache(maxsize=1)
def _get_nc():
    return _build_nc()


def _make_in_maps(x, attention_mask, W_qkv, b_qkv, W_out, b_out):
    bf16 = ml_dtypes.bfloat16
    x = np.asarray(x, dtype=np.float32).reshape(R, D)
    xT = np.ascontiguousarray(x.T).astype(bf16)
    W_qkv = np.asarray(W_qkv, dtype=np.float32)
    W_out = np.asarray(W_out, dtype=np.float32)
    b_qkv = np.asarray(b_qkv, dtype=np.float32)
    b_out = np.asarray(b_out, dtype=np.float32)
    mask = np.asarray(attention_mask).astype(np.int32)
    maskT = np.ascontiguousarray(
        mask.reshape(B, NKT, 128).transpose(2, 0, 1).reshape(128, B * NKT)
    )

    def _ktile(w):  # [1024, 128] -> [128(p), 8(t), 128(m)]
        return np.ascontiguousarray(
            w.reshape(KT8, 128, 128).transpose(1, 0, 2)).astype(bf16)

    in_maps = []
    for c in range(CORES):
        s = slice(128 * c, 128 * (c + 1))
        in_maps.append({
            "xT": xT,
            "wq": _ktile(W_qkv[:, s]),
            "wk": _ktile(W_qkv[:, D:][:, s]),
            "wv": _ktile(W_qkv[:, 2 * D:][:, s]),
            "wo": np.ascontiguousarray(W_out[s, :]).astype(bf16),
            "bqkv": np.ascontiguousarray(
                np.stack([b_qkv[s], b_qkv[D:][s], b_qkv[2 * D:][s]])
            ),
            "maskT": maskT,
        })
    return in_maps


def timeline_estimate_ns():
    """Cost-model makespan of the per-core program (no HW needed)."""
    from concourse.timeline_sim import TimelineSim
    return TimelineSim(_get_nc(), trace=False).simulate()


def run(trace=False, **inputs):
    nc = _get_nc()
    b_out = np.asarray(inputs["b_out"], dtype=np.float32)
    in_maps = _make_in_maps(**inputs)
    try:
        res = run_bass_kernel_spmd(
            nc, in_maps, core_ids=list(range(CORES)), trace=trace,
        )
    except (ImportError, ModuleNotFoundError):
        # NTFF profiling hook unavailable in this client image
        res = run_bass_kernel_spmd(
            nc, in_maps, core_ids=list(range(CORES)), trace=False,
        )
    acc = np.zeros((D, R), dtype=np.float32)
    for r in res.results:
        acc += r["outT"]
    out = (np.ascontiguousarray(acc.T) + b_out).reshape(B, N, D)
    return out, res


def kernel(**inputs):
    out, _ = run(trace=False, **inputs)
    return out



# revision 25
# speedup vs baseline: 1.1419x; 1.1419x over previous
"""Multi-head attention (B=2, N=2048, D=1024, H=16) on 8 trn2 NeuronCores.

Sharding: tensor-parallel over heads - core c owns heads (2c, 2c+1); host sums
the 8 full-size partial outputs (row-sharded W_out product) and transposes.

Device dataflow per core (fp8 DoubleRow matmuls where precision allows):
  phase 1 (QKV projections, 0.75 cyc/row): weights are host-split into
    (hi, lo) fp8e4 pairs; x is host-split into x_hi/x_lo fp8e4.
    Per k-tile: one DR matmul (w_hi,w_lo)x(x_hi bcast) + per k-tile-pair one
    DR matmul (w_hi,w_hi)x(x_lo pair) -> full-term fp32 PSUM (drops only the
    O(2^-8) lo*lo term, bf16-class accuracy).
    q -> fp8 (one-sided), k -> (hi,lo) fp8 pair, v -> bf16 natural [keys,dims]
    with a cd*mask denominator column appended.
  phase 2 (attention): scoresT = (k_hi,k_lo) DR-matmul (q,q) at 0.5 cyc/row
    (k at full precision, q one-sided fp8).  exp on ACT engine (bf16 out),
    a fraction of k-tiles on DVE via a custom 8-stage squaring op
    ((1+t/8+t^2/128)^8).  ctx natural orientation: out[queries,65] bf16
    matmuls filling all 128 output partitions (65 cyc/instr); denominators
    ride along as column 64 (cd*mask); gpsimd normalize_recip; DMA-transpose
    [128q, 2hx64d] -> ctxT.
  phase 3: outT_partial = wo^T ctxT (bf16), partials DMA'd as bf16; host sums.
"""

import os
import sys
import functools

import numpy as np
import ml_dtypes

for _p in (
    "/root/.axon_site",
    "/root/.axon_site/_ro/trn_rl_repo",
    "/root/.axon_site/_ro/pypackages",
    "/opt/trn_rl_repo",
):
    if os.path.isdir(_p) and _p not in sys.path:
        sys.path.append(_p)

def _ensure_axon():
    """Register the axon PJRT backend if sitecustomize didn't run."""
    import jax
    try:
        backends = jax._src.xla_bridge._backend_factories
        if "axon" in backends:
            return
    except Exception:
        pass
    try:
        from trn_agent_boot.trn_boot import boot
        os.environ.setdefault("AXON_POOL_SVC_OVERRIDE", "127.0.0.1")
        os.environ.setdefault("AXON_LOOPBACK_RELAY", "1")
        boot(os.environ["TRN_TERMINAL_PRECOMPUTED_JSON"],
             "/opt/axon/libaxon_pjrt.so")
    except Exception:
        pass


_ensure_axon()

import concourse.tile as tile
from concourse import bacc, mybir
from concourse.alu_op_type import AluOpType
from concourse.bass_utils import run_bass_kernel_spmd

BF16 = mybir.dt.bfloat16
F8 = mybir.dt.float8e4
F32 = mybir.dt.float32
DR = mybir.MatmulPerfMode.DoubleRow

B, N, D, H = 2, 2048, 1024, 16
DK = D // H            # 64
CORES = 8
HPC = H // CORES       # 2 heads per core
R = B * N              # 4096 rows total
RB = N                 # rows per batch
KT8 = D // 128         # 8 k-tiles over the model dim
QC = RB // 512         # 4 query chunks of 512 per batch
NKT = RB // 128        # 16 key tiles of 128 per batch
OT = D // 128          # 8 output-dim tiles

SW = 32.0              # weight pre-scale before fp8 split
CD = 1.0 / 8.0         # denominator column constant
EXP_SCALE = (DK ** -0.5) / (SW * SW)
OUT_SCALE = SW * SW / CD          # psum partial = OUT_SCALE * true partial
import os as _os
DVE_EXP_KTS = tuple(int(x) for x in _os.environ.get(
    "K_DVE_KTS", "2,6,10,14").split(",") if x != "")
QK_ENG = _os.environ.get("K_QK_ENG", "v")      # q/k psum->sbuf copy engine
CTXF_ENG = _os.environ.get("K_CTXF_ENG", "v")  # ctx psum->sbuf copy engine
P3_ENGS = _os.environ.get("K_P3_ENGS", "svsvsvsv")

# ---------------------------------------------------------------- custom exp
from concourse import dve_ops as _dve_ops
from concourse.dve_spec import AluOp, Bin, Spec, Src0, One, C0, lower as _dve_lower
from concourse.dve_spec import _has_src1
from concourse.dve_uop import DveOpSpec


def _exp_sq3_ref(in0, in1, s0, s1, imm2):
    d = in0.astype(np.float32) * np.float32(s0)
    e = d + np.float32(1.0)
    base = e * e + d * d
    r1 = base * base
    r2 = r1 * r1
    return r2 * r2


def _register_exp_op():
    name = "EXP_SQ3_ANT"
    if name in _dve_ops._SUB_OPCODE_FOR_NAME:
        return next(o for o in _dve_ops.OPS if o.name == name)
    d = Bin(AluOp.MULTIPLY, Src0, C0)
    e = Bin(AluOp.ADD, d, One)
    base = Bin(AluOp.ADD, Bin(AluOp.MULTIPLY, e, e), Bin(AluOp.MULTIPLY, d, d))
    r1 = Bin(AluOp.MULTIPLY, base, base)
    r2 = Bin(AluOp.MULTIPLY, r1, r1)
    r3 = Bin(AluOp.MULTIPLY, r2, r2)
    spec = Spec(body=r3, reference=_exp_sq3_ref)
    _dve_ops._SUB_OPCODE_FOR_NAME[name] = 17
    shas = {}
    for ver in ("v3", "v4"):
        dspec = DveOpSpec(name=name, opcode=17, uops=_dve_lower(spec, ver=ver),
                          rd1_en=_has_src1(spec))
        shas[ver] = dspec.sha(ver)
    op = _dve_ops.DveOp(name, spec, subdim=False, uops_sha=shas)
    _dve_ops.OPS.append(op)
    _dve_ops.CUSTOM_DVE_SPECS[name] = spec
    return op


EXP_OP = _register_exp_op()


# ------------------------------------------------------------------- program
def _build_nc():
    nc = bacc.Bacc(
        "TRN2", target_bir_lowering=False, debug=False, enable_asserts=False,
        num_devices=CORES,
    )

    xhi_d = nc.dram_tensor("xhi", [128, KT8, R], F8, kind="ExternalInput").ap()
    xlo_d = nc.dram_tensor("xlo", [128, KT8, R], F8, kind="ExternalInput").ap()
    # (hi, lo) weight pairs + hi-only tensors, per projection
    wqp_d = nc.dram_tensor("wqp", [128, KT8, 2, 128], F8, kind="ExternalInput").ap()
    wkp_d = nc.dram_tensor("wkp", [128, KT8, 2, 128], F8, kind="ExternalInput").ap()
    wvp_d = nc.dram_tensor("wvp", [128, KT8, 2, 128], F8, kind="ExternalInput").ap()
    wqh_d = nc.dram_tensor("wqh", [128, KT8, 128], F8, kind="ExternalInput").ap()
    wkh_d = nc.dram_tensor("wkh", [128, KT8, 128], F8, kind="ExternalInput").ap()
    wvh_d = nc.dram_tensor("wvh", [128, KT8, 128], F8, kind="ExternalInput").ap()
    wo_d = nc.dram_tensor("wo", [128, D], BF16, kind="ExternalInput").ap()
    bqkv_d = nc.dram_tensor("bqkv", [3, 128], F32, kind="ExternalInput").ap()
    maskv_d = nc.dram_tensor("maskv", [128, B * NKT], F32, kind="ExternalInput").ap()
    maskcd_d = nc.dram_tensor("maskcd", [128, B * NKT], F32, kind="ExternalInput").ap()
    outT_d = nc.dram_tensor("outT", [D, R], BF16, kind="ExternalOutput").ap()

    with tile.TileContext(nc) as tc:
        with (
            tc.tile_pool(name="persist", bufs=1) as persist,
            tc.tile_pool(name="xt", bufs=3) as xt_pool,
            tc.tile_pool(name="aw", bufs=2) as aw_pool,
            tc.tile_pool(name="small", bufs=4) as small,
            tc.tile_pool(name="outsb", bufs=2) as outsb_pool,
            tc.tile_pool(name="psmm", bufs=2, space="PSUM") as psmm_pool,
            tc.tile_pool(name="pssc", bufs=2, space="PSUM") as pssc_pool,
            tc.tile_pool(name="psctx", bufs=2, space="PSUM") as psctx_pool,
        ):
            # ---- weights / consts ----
            wqp = persist.tile([128, KT8, 2, 128], F8, tag="wqp")
            wkp = persist.tile([128, KT8, 2, 128], F8, tag="wkp")
            wvp = persist.tile([128, KT8, 2, 128], F8, tag="wvp")
            wqh = persist.tile([128, KT8, 128], F8, tag="wqh")
            wkh = persist.tile([128, KT8, 128], F8, tag="wkh")
            wvh = persist.tile([128, KT8, 128], F8, tag="wvh")
            wo_sb = persist.tile([128, D], BF16, tag="wo")
            bq_sb = persist.tile([128, 1], F32, tag="bq")
            bk_sb = persist.tile([128, 1], F32, tag="bk")
            bv_rep = persist.tile([128, 128], F32, tag="bv")
            maskv = persist.tile([128, B * NKT], F32, tag="maskv")
            maskcd = persist.tile([128, B * NKT], F32, tag="maskcd")

            def load_consts():
                # order: first-needed first (wqp/wqh feed the very first matmul)
                for w_sb, w_d in ((wqp, wqp_d), (wqh, wqh_d),
                                  (wkp, wkp_d), (wkh, wkh_d),
                                  (wvp, wvp_d), (wvh, wvh_d),
                                  (wo_sb, wo_d)):
                    nc.sync.dma_start(out=w_sb, in_=w_d)
                nc.sync.dma_start(
                    out=bq_sb, in_=bqkv_d[0:1, :].rearrange("o p -> p o"))
                nc.sync.dma_start(
                    out=bk_sb, in_=bqkv_d[1:2, :].rearrange("o p -> p o"))
                nc.sync.dma_start(
                    out=bv_rep, in_=bqkv_d[2:3, :].partition_broadcast(128))
                nc.sync.dma_start(out=maskv, in_=maskv_d)
                nc.sync.dma_start(out=maskcd, in_=maskcd_d)

            # ---- persistent activations ----
            qt = [persist.tile([128, RB], F8, tag=f"qt{b}", name=f"qt{b}")
                  for b in range(B)]
            kt2 = [persist.tile([128, 2, RB], F8, tag=f"kt{b}", name=f"kt{b}")
                   for b in range(B)]
            # V natural [keys, kt, h, 66]: cols 0-63 v, col 64 cd*mask
            v_sb = [persist.tile([128, NKT, HPC, 66], BF16, tag=f"v{b}",
                                 name=f"v{b}") for b in range(B)]
            ctxT = [persist.tile([128, RB], BF16, tag=f"ctxT{b}",
                                 name=f"ctxT{b}") for b in range(B)]

            def fill_cd_cols(b):
                # v ones-columns: cd * mask, broadcast over heads
                nc.gpsimd.tensor_copy(
                    out=v_sb[b][:, :, :, 64:65].squeeze(3),
                    in_=maskcd[:, b * NKT:(b + 1) * NKT]
                        .unsqueeze(2).broadcast_to([128, NKT, HPC]),
                )

            # ---- phase 1: QKV projections (fp8 DR, 3-term) ----
            def proj_matmuls(ps, xhi_t, xlo_t, w_pair, w_hi, n0, n1,
                             w_is_lhs):
                """12 DR matmuls accumulating x @ w into ps[128, n1-n0]."""
                first = True
                for kt in range(KT8):
                    if w_is_lhs:
                        nc.tensor.matmul(
                            ps, lhsT=w_pair[:, kt, :, :],
                            rhs=xhi_t[:, kt, n0:n1].unsqueeze(1)
                                .broadcast_to([128, 2, n1 - n0]),
                            start=first, stop=False, perf_mode=DR)
                    else:
                        nc.tensor.matmul(
                            ps, lhsT=xhi_t[:, kt, n0:n1].unsqueeze(1)
                                .broadcast_to([128, 2, n1 - n0]),
                            rhs=w_pair[:, kt, :, :],
                            start=first, stop=False, perf_mode=DR)
                    first = False
                for tp in range(KT8 // 2):
                    last = tp == KT8 // 2 - 1
                    if w_is_lhs:
                        nc.tensor.matmul(
                            ps, lhsT=w_hi[:, 2 * tp:2 * tp + 2, :],
                            rhs=xlo_t[:, 2 * tp:2 * tp + 2, n0:n1],
                            start=False, stop=last, perf_mode=DR)
                    else:
                        nc.tensor.matmul(
                            ps, lhsT=xlo_t[:, 2 * tp:2 * tp + 2, n0:n1],
                            rhs=w_hi[:, 2 * tp:2 * tp + 2, :],
                            start=False, stop=last, perf_mode=DR)

            def phase1_dma(b, chb):
                roff = b * RB + chb * 512
                xhi_t = xt_pool.tile([128, KT8, 512], F8, tag="xhi")
                xlo_t = xt_pool.tile([128, KT8, 512], F8, tag="xlo")
                nc.sync.dma_start(out=xhi_t, in_=xhi_d[:, :, roff:roff + 512])
                nc.sync.dma_start(out=xlo_t, in_=xlo_d[:, :, roff:roff + 512])
                return xhi_t, xlo_t

            def phase1_chunk(b, chb, pre=None):
                cols = slice(chb * 512, (chb + 1) * 512)
                xhi_t, xlo_t = pre if pre is not None else phase1_dma(b, chb)
                # Q
                qk_eng = nc.vector if QK_ENG == "v" else nc.scalar
                ps = psmm_pool.tile([128, 512], F32, tag="mm", name="qps")
                proj_matmuls(ps, xhi_t, xlo_t, wqp, wqh, 0, 512, True)
                qk_eng.tensor_scalar_add(
                    out=qt[b][:, cols], in0=ps, scalar1=bq_sb)
                # K -> hi/lo pair
                ps = psmm_pool.tile([128, 512], F32, tag="mm", name="kps")
                proj_matmuls(ps, xhi_t, xlo_t, wkp, wkh, 0, 512, True)
                qk_eng.tensor_scalar_add(
                    out=kt2[b][:, 0, cols], in0=ps, scalar1=bk_sb)
                qk_eng.scalar_tensor_tensor(
                    out=kt2[b][:, 1, cols], in0=ps, scalar=bk_sb,
                    in1=kt2[b][:, 0, cols], op0=AluOpType.add,
                    op1=AluOpType.subtract)
                # V (natural): out rows = x columns -> 4 sub-tiles of 128
                for sub in range(4):
                    rt = chb * 4 + sub
                    n0 = sub * 128
                    vps = psmm_pool.tile([128, 128], F32, tag="mm", name="vps")
                    proj_matmuls(vps, xhi_t, xlo_t, wvp, wvh, n0, n0 + 128,
                                 False)
                    mcol = maskv[:, b * NKT + rt:b * NKT + rt + 1]
                    for h in range(HPC):
                        vd = v_sb[b][:, rt, h, 0:64]
                        nc.vector.tensor_add(
                            out=vd, in0=vps[:, h * 64:(h + 1) * 64],
                            in1=bv_rep[:, h * 64:(h + 1) * 64])
                        nc.gpsimd.tensor_scalar_mul(
                            out=vd, in0=vd, scalar1=mcol)

            # ---- phase 2: attention ----
            def attn_alloc():
                # ctx accumulators: [128, 4, 128] x2 (h), slot j = qsub
                return [psctx_pool.tile([128, 4, 128], F32, tag="ctx",
                                        name=f"ctxps{h}") for h in range(HPC)]

            def attn_ctx(b, cps, awt, kt):
                # one start per PSUM tile: start zeroing covers the whole
                # tile, so only (kt0, j0) starts; other regions accumulate
                # onto the zeroed tile
                for h in range(HPC):
                    for j in range(4):
                        nc.tensor.matmul(
                            cps[h][:, j, 0:65],
                            lhsT=awt[:, kt, h, j * 128:(j + 1) * 128],
                            rhs=v_sb[b][:, kt, h, 0:65],
                            start=(kt == 0 and j == 0),
                            stop=(kt == NKT - 1),
                            skip_group_check=True)

            def attn_kts(b, qc, cps, kts, pend):
                qs = qc * 512
                for kt in kts:
                    ks = kt * 128
                    sc = pssc_pool.tile([128, HPC, 512], F32, tag="sc",
                                        name="scps")
                    for h in range(HPC):
                        p0 = h * 64
                        nc.tensor.matmul(
                            sc[:, h, :],
                            lhsT=kt2[b][p0:p0 + 64, :, ks:ks + 128],
                            rhs=qt[b][p0:p0 + 64, qs:qs + 512].unsqueeze(1)
                                .broadcast_to([64, 2, 512]),
                            start=True, stop=True, perf_mode=DR,
                            tile_position=(p0, 0))
                    awt = aw_pool.tile([128, NKT, HPC, 512], BF16, tag="aw",
                                       name="awt") if kt == 0 else attn_kts.awt
                    attn_kts.awt = awt
                    if kt in DVE_EXP_KTS:
                        nc.vector._custom_dve(
                            EXP_OP, out=awt[:, kt, :, :], in0=sc,
                            s0=EXP_SCALE / 16.0)
                    else:
                        nc.scalar.activation(
                            out=awt[:, kt, :, :], in_=sc,
                            func=mybir.ActivationFunctionType.Exp,
                            scale=EXP_SCALE)
                    # software pipeline: run the PREVIOUS kt's ctx now, so the
                    # PE never blocks on this kt's exp round-trip
                    while len(pend) >= 2:
                        attn_ctx(b, cps, awt, pend.pop(0))
                    pend.append(kt)

            def attn_finish(b, qc, cps):
                ctxf = small.tile([128, HPC, 4, 65], F32, tag="ctxf",
                                  name="ctxf")
                ctxf_eng = nc.vector if CTXF_ENG == "v" else nc.gpsimd
                for h in range(HPC):
                    ctxf_eng.tensor_copy(
                        out=ctxf[:, h, :, :], in_=cps[h][:, :, 0:65])
                ctxn = small.tile([128, 4, HPC, 64], BF16, tag="ctxn",
                                  name="ctxn")
                for h in range(HPC):
                    for j in range(4):
                        nc.gpsimd.normalize_recip(
                            ctxn[:, j, h, :], ctxf[:, h, j, 0:64],
                            ctxf[:, h, j, 64:65])
                for j in range(4):
                    qs = b * 0 + qc * 512 + j * 128
                    nc.sync.dma_start_transpose(
                        out=ctxT[b][:, qs:qs + 128],
                        in_=ctxn[:, j, :, :])

            # ---- phase 3 ----
            # copy engines: nc.vector / nc.gpsimd / nc.scalar mix per ot
            p3_osb = {}

            def phase3_half(b, rc, half, engs=None):
                engs = engs or (P3_ENGS[:4] if half == 0 else P3_ENGS[4:])
                emap = {"v": nc.vector, "g": nc.gpsimd, "s": nc.scalar}
                if half == 0:
                    p3_osb[(b, rc)] = outsb_pool.tile(
                        [128, OT, 512], BF16, tag="osb", name=f"osb{b}{rc}")
                osb = p3_osb[(b, rc)]
                for i in range(4):
                    ot = half * 4 + i
                    ps = psmm_pool.tile([128, 512], F32, tag="mm",
                                        name="outps")
                    nc.tensor.matmul(
                        ps, lhsT=wo_sb[:, ot * 128:(ot + 1) * 128],
                        rhs=ctxT[b][:, rc * 512:(rc + 1) * 512],
                        start=True, stop=True)
                    eng = emap[engs[i]]
                    if eng is nc.scalar:
                        eng.copy(out=osb[:, ot, :], in_=ps)
                    else:
                        eng.tensor_copy(out=osb[:, ot, :], in_=ps)
                if half == 1:
                    cs = b * RB + rc * 512
                    outT_r = outT_d.rearrange("(t p) r -> p t r", p=128)
                    nc.sync.dma_start(out=outT_r[:, :, cs:cs + 512], in_=osb)

            def phase3(b, rc, engs=None):
                engs = engs or P3_ENGS
                phase3_half(b, rc, 0, engs[:4])
                phase3_half(b, rc, 1, engs[4:])

            # ---- schedule ----
            def attn(b, qc, hooks=None):
                cps = attn_alloc()
                pend = []
                lo = 0
                for at in sorted(hooks or {}):
                    attn_kts(b, qc, cps, list(range(lo, at)), pend)
                    hooks[at]()
                    lo = at
                attn_kts(b, qc, cps, list(range(lo, NKT)), pend)
                while pend:
                    attn_ctx(b, cps, attn_kts.awt, pend.pop(0))
                attn_finish(b, qc, cps)

            pre00 = phase1_dma(0, 0)
            load_consts()
            fill_cd_cols(0)
            fill_cd_cols(1)
            phase1_chunk(0, 0, pre=pre00)
            attn(0, 0, {4: lambda: phase1_chunk(0, 1),
                        8: lambda: phase1_chunk(0, 2),
                        12: lambda: phase1_chunk(0, 3)})
            attn(0, 1, {4: lambda: phase1_chunk(1, 0),
                        12: lambda: phase1_chunk(1, 1)})
            attn(0, 2, {4: lambda: phase1_chunk(1, 2),
                        12: lambda: phase3_half(0, 0, 0)})
            attn(0, 3, {4: lambda: phase3_half(0, 0, 1),
                        8: lambda: phase1_chunk(1, 3),
                        12: lambda: phase3_half(0, 1, 0)})
            attn(1, 0, {4: lambda: phase3_half(0, 1, 1),
                        12: lambda: phase3_half(0, 2, 0)})
            attn(1, 1, {4: lambda: phase3_half(0, 2, 1),
                        12: lambda: phase3_half(0, 3, 0)})
            attn(1, 2, {4: lambda: phase3_half(0, 3, 1),
                        12: lambda: phase3_half(1, 0, 0)})
            attn(1, 3, {4: lambda: phase3_half(1, 0, 1),
                        8: lambda: phase3_half(1, 1, 0),
                        12: lambda: phase3_half(1, 1, 1)})
            phase3(1, 2, engs="svsvsvsv")
            phase3(1, 3, engs="svsvsvsv")

    nc.compile()
    return nc


@functools.lru_cache(maxsize=1)
def _get_nc():
    return _build_nc()


def _split8(a):
    f8 = ml_dtypes.float8_e4m3
    hi = a.astype(f8)
    lo = (a - hi.astype(np.float32)).astype(f8)
    return hi, lo


def _make_in_maps(x, attention_mask, W_qkv, b_qkv, W_out, b_out):
    bf16 = ml_dtypes.bfloat16
    x = np.asarray(x, dtype=np.float32).reshape(R, D)
    xT = np.ascontiguousarray(x.T)                     # [D, R]
    xk = np.ascontiguousarray(
        xT.reshape(KT8, 128, R).transpose(1, 0, 2))    # [128, KT8, R]
    x_hi, x_lo = _split8(xk)
    W_qkv = np.asarray(W_qkv, dtype=np.float32)
    W_out = np.asarray(W_out, dtype=np.float32)
    b_qkv = np.asarray(b_qkv, dtype=np.float32)
    mask = np.asarray(attention_mask).astype(np.float32)
    maskv = np.ascontiguousarray(
        mask.reshape(B, NKT, 128).transpose(2, 0, 1).reshape(128, B * NKT))
    maskcd = np.ascontiguousarray(maskv * CD)

    def _wsplit(w):  # [1024, 128] -> pair [128, KT8, 2, 128] + hi [128, KT8, 128]
        wk = np.ascontiguousarray(
            (w * SW).reshape(KT8, 128, 128).transpose(1, 0, 2))
        hi, lo = _split8(wk)
        pair = np.ascontiguousarray(
            np.stack([hi, lo], axis=2))                # [128, KT8, 2, 128]
        return pair, np.ascontiguousarray(hi)

    in_maps = []
    for c in range(CORES):
        s = slice(128 * c, 128 * (c + 1))
        wqp, wqh = _wsplit(W_qkv[:, s])
        wkp, wkh = _wsplit(W_qkv[:, D:][:, s])
        wvp, wvh = _wsplit(W_qkv[:, 2 * D:][:, s])
        in_maps.append({
            "xhi": x_hi, "xlo": x_lo,
            "wqp": wqp, "wkp": wkp, "wvp": wvp,
            "wqh": wqh, "wkh": wkh, "wvh": wvh,
            "wo": np.ascontiguousarray(W_out[s, :] * SW).astype(bf16),
            "bqkv": np.ascontiguousarray(
                np.stack([b_qkv[s], b_qkv[D:][s], b_qkv[2 * D:][s]]) * SW),
            "maskv": maskv,
            "maskcd": maskcd,
        })
    return in_maps


def timeline_estimate_ns():
    """Cost-model makespan of the per-core program (no HW needed)."""
    from concourse.timeline_sim import TimelineSim
    return TimelineSim(_get_nc(), trace=False).simulate()


def run(trace=False, **inputs):
    nc = _get_nc()
    b_out = np.asarray(inputs["b_out"], dtype=np.float32)
    in_maps = _make_in_maps(**inputs)
    try:
        res = run_bass_kernel_spmd(
            nc, in_maps, core_ids=list(range(CORES)), trace=trace,
        )
    except (ImportError, ModuleNotFoundError):
        res = run_bass_kernel_spmd(
            nc, in_maps, core_ids=list(range(CORES)), trace=False,
        )
    acc = np.zeros((D, R), dtype=np.float32)
    for r in res.results:
        acc += r["outT"].astype(np.float32)
    out = (np.ascontiguousarray(acc.T) / OUT_SCALE + b_out).reshape(B, N, D)
    return out, res


def kernel(**inputs):
    out, _ = run(trace=False, **inputs)
    return out


# revision 26
# speedup vs baseline: 1.1428x; 1.0008x over previous
"""Multi-head attention (B=2, N=2048, D=1024, H=16) on 8 trn2 NeuronCores.

Sharding: tensor-parallel over heads - core c owns heads (2c, 2c+1); host sums
the 8 full-size partial outputs (row-sharded W_out product) and transposes.

Device dataflow per core (fp8 DoubleRow matmuls where precision allows):
  phase 1 (QKV projections, 0.75 cyc/row): weights are host-split into
    (hi, lo) fp8e4 pairs; x is host-split into x_hi/x_lo fp8e4.
    Per k-tile: one DR matmul (w_hi,w_lo)x(x_hi bcast) + per k-tile-pair one
    DR matmul (w_hi,w_hi)x(x_lo pair) -> full-term fp32 PSUM (drops only the
    O(2^-8) lo*lo term, bf16-class accuracy).
    q -> fp8 (one-sided), k -> (hi,lo) fp8 pair, v -> bf16 natural [keys,dims]
    with a cd*mask denominator column appended.
  phase 2 (attention): scoresT = (k_hi,k_lo) DR-matmul (q,q) at 0.5 cyc/row
    (k at full precision, q one-sided fp8).  exp on ACT engine (bf16 out),
    a fraction of k-tiles on DVE via a custom 8-stage squaring op
    ((1+t/8+t^2/128)^8).  ctx natural orientation: out[queries,65] bf16
    matmuls filling all 128 output partitions (65 cyc/instr); denominators
    ride along as column 64 (cd*mask); gpsimd normalize_recip; DMA-transpose
    [128q, 2hx64d] -> ctxT.
  phase 3: outT_partial = wo^T ctxT (bf16), partials DMA'd as bf16; host sums.
"""

import os
import sys
import functools

import numpy as np
import ml_dtypes

for _p in (
    "/root/.axon_site",
    "/root/.axon_site/_ro/trn_rl_repo",
    "/root/.axon_site/_ro/pypackages",
    "/opt/trn_rl_repo",
):
    if os.path.isdir(_p) and _p not in sys.path:
        sys.path.append(_p)

def _ensure_axon():
    """Register the axon PJRT backend if sitecustomize didn't run."""
    import jax
    try:
        backends = jax._src.xla_bridge._backend_factories
        if "axon" in backends:
            return
    except Exception:
        pass
    try:
        from trn_agent_boot.trn_boot import boot
        os.environ.setdefault("AXON_POOL_SVC_OVERRIDE", "127.0.0.1")
        os.environ.setdefault("AXON_LOOPBACK_RELAY", "1")
        boot(os.environ["TRN_TERMINAL_PRECOMPUTED_JSON"],
             "/opt/axon/libaxon_pjrt.so")
    except Exception:
        pass


_ensure_axon()

import concourse.tile as tile
from concourse import bacc, mybir
from concourse.alu_op_type import AluOpType
from concourse.bass_utils import run_bass_kernel_spmd

BF16 = mybir.dt.bfloat16
F8 = mybir.dt.float8e4
F32 = mybir.dt.float32
DR = mybir.MatmulPerfMode.DoubleRow

B, N, D, H = 2, 2048, 1024, 16
DK = D // H            # 64
CORES = 8
HPC = H // CORES       # 2 heads per core
R = B * N              # 4096 rows total
RB = N                 # rows per batch
KT8 = D // 128         # 8 k-tiles over the model dim
QC = RB // 512         # 4 query chunks of 512 per batch
NKT = RB // 128        # 16 key tiles of 128 per batch
OT = D // 128          # 8 output-dim tiles

SW = 32.0              # weight pre-scale before fp8 split
CD = 1.0 / 8.0         # denominator column constant
EXP_SCALE = (DK ** -0.5) / (SW * SW)
OUT_SCALE = SW * SW / CD          # psum partial = OUT_SCALE * true partial
import os as _os
DVE_EXP_KTS = tuple(int(x) for x in _os.environ.get(
    "K_DVE_KTS", "2,6,10,14").split(",") if x != "")
QK_ENG = _os.environ.get("K_QK_ENG", "v")      # q/k psum->sbuf copy engine
CTXF_ENG = _os.environ.get("K_CTXF_ENG", "v")  # ctx psum->sbuf copy engine
P3_ENGS = _os.environ.get("K_P3_ENGS", "svsvsvsv")

# ---------------------------------------------------------------- custom exp
from concourse import dve_ops as _dve_ops
from concourse.dve_spec import AluOp, Bin, Spec, Src0, One, C0, lower as _dve_lower
from concourse.dve_spec import _has_src1
from concourse.dve_uop import DveOpSpec


def _exp_sq3_ref(in0, in1, s0, s1, imm2):
    d = in0.astype(np.float32) * np.float32(s0)
    e = d + np.float32(1.0)
    base = e * e + d * d
    r1 = base * base
    r2 = r1 * r1
    return r2 * r2


def _register_exp_op():
    name = "EXP_SQ3_ANT"
    if name in _dve_ops._SUB_OPCODE_FOR_NAME:
        return next(o for o in _dve_ops.OPS if o.name == name)
    d = Bin(AluOp.MULTIPLY, Src0, C0)
    e = Bin(AluOp.ADD, d, One)
    base = Bin(AluOp.ADD, Bin(AluOp.MULTIPLY, e, e), Bin(AluOp.MULTIPLY, d, d))
    r1 = Bin(AluOp.MULTIPLY, base, base)
    r2 = Bin(AluOp.MULTIPLY, r1, r1)
    r3 = Bin(AluOp.MULTIPLY, r2, r2)
    spec = Spec(body=r3, reference=_exp_sq3_ref)
    _dve_ops._SUB_OPCODE_FOR_NAME[name] = 17
    shas = {}
    for ver in ("v3", "v4"):
        dspec = DveOpSpec(name=name, opcode=17, uops=_dve_lower(spec, ver=ver),
                          rd1_en=_has_src1(spec))
        shas[ver] = dspec.sha(ver)
    op = _dve_ops.DveOp(name, spec, subdim=False, uops_sha=shas)
    _dve_ops.OPS.append(op)
    _dve_ops.CUSTOM_DVE_SPECS[name] = spec
    return op


EXP_OP = _register_exp_op()


# ------------------------------------------------------------------- program
def _build_nc():
    nc = bacc.Bacc(
        "TRN2", target_bir_lowering=False, debug=False, enable_asserts=False,
        num_devices=CORES,
    )

    xhi_d = nc.dram_tensor("xhi", [128, KT8, R], F8, kind="ExternalInput").ap()
    xlo_d = nc.dram_tensor("xlo", [128, KT8, R], F8, kind="ExternalInput").ap()
    # (hi, lo) weight pairs + hi-only tensors, per projection
    wqp_d = nc.dram_tensor("wqp", [128, KT8, 2, 128], F8, kind="ExternalInput").ap()
    wkp_d = nc.dram_tensor("wkp", [128, KT8, 2, 128], F8, kind="ExternalInput").ap()
    wvp_d = nc.dram_tensor("wvp", [128, KT8, 2, 128], F8, kind="ExternalInput").ap()
    wqh_d = nc.dram_tensor("wqh", [128, KT8, 128], F8, kind="ExternalInput").ap()
    wkh_d = nc.dram_tensor("wkh", [128, KT8, 128], F8, kind="ExternalInput").ap()
    wvh_d = nc.dram_tensor("wvh", [128, KT8, 128], F8, kind="ExternalInput").ap()
    wo_d = nc.dram_tensor("wo", [128, D], BF16, kind="ExternalInput").ap()
    bqkv_d = nc.dram_tensor("bqkv", [3, 128], F32, kind="ExternalInput").ap()
    maskv_d = nc.dram_tensor("maskv", [128, B * NKT], F32, kind="ExternalInput").ap()
    maskcd_d = nc.dram_tensor("maskcd", [128, B * NKT], F32, kind="ExternalInput").ap()
    outT_d = nc.dram_tensor("outT", [D, R], BF16, kind="ExternalOutput").ap()

    with tile.TileContext(nc) as tc:
        with (
            tc.tile_pool(name="persist", bufs=1) as persist,
            tc.tile_pool(name="xt", bufs=3) as xt_pool,
            tc.tile_pool(name="aw", bufs=2) as aw_pool,
            tc.tile_pool(name="small", bufs=4) as small,
            tc.tile_pool(name="outsb", bufs=2) as outsb_pool,
            tc.tile_pool(name="psmm", bufs=2, space="PSUM") as psmm_pool,
            tc.tile_pool(name="pssc", bufs=2, space="PSUM") as pssc_pool,
            tc.tile_pool(name="psctx", bufs=2, space="PSUM") as psctx_pool,
        ):
            # ---- weights / consts ----
            wqp = persist.tile([128, KT8, 2, 128], F8, tag="wqp")
            wkp = persist.tile([128, KT8, 2, 128], F8, tag="wkp")
            wvp = persist.tile([128, KT8, 2, 128], F8, tag="wvp")
            wqh = persist.tile([128, KT8, 128], F8, tag="wqh")
            wkh = persist.tile([128, KT8, 128], F8, tag="wkh")
            wvh = persist.tile([128, KT8, 128], F8, tag="wvh")
            wo_sb = persist.tile([128, D], BF16, tag="wo")
            bq_sb = persist.tile([128, 1], F32, tag="bq")
            bk_sb = persist.tile([128, 1], F32, tag="bk")
            bv_rep = persist.tile([128, 128], F32, tag="bv")
            maskv = persist.tile([128, B * NKT], F32, tag="maskv")
            maskcd = persist.tile([128, B * NKT], F32, tag="maskcd")

            def load_consts():
                # order: first-needed first (wqp/wqh feed the very first matmul)
                for w_sb, w_d in ((wqp, wqp_d), (wqh, wqh_d),
                                  (wkp, wkp_d), (wkh, wkh_d),
                                  (wvp, wvp_d), (wvh, wvh_d),
                                  (wo_sb, wo_d)):
                    nc.sync.dma_start(out=w_sb, in_=w_d)
                nc.sync.dma_start(
                    out=bq_sb, in_=bqkv_d[0:1, :].rearrange("o p -> p o"))
                nc.sync.dma_start(
                    out=bk_sb, in_=bqkv_d[1:2, :].rearrange("o p -> p o"))
                nc.sync.dma_start(
                    out=bv_rep, in_=bqkv_d[2:3, :].partition_broadcast(128))
                nc.sync.dma_start(out=maskv, in_=maskv_d)
                nc.sync.dma_start(out=maskcd, in_=maskcd_d)

            # ---- persistent activations ----
            qt = [persist.tile([128, RB], F8, tag=f"qt{b}", name=f"qt{b}")
                  for b in range(B)]
            kt2 = [persist.tile([128, 2, RB], F8, tag=f"kt{b}", name=f"kt{b}")
                   for b in range(B)]
            # V natural [keys, kt, h, 66]: cols 0-63 v, col 64 cd*mask
            v_sb = [persist.tile([128, NKT, HPC, 66], BF16, tag=f"v{b}",
                                 name=f"v{b}") for b in range(B)]
            ctxT = [persist.tile([128, RB], BF16, tag=f"ctxT{b}",
                                 name=f"ctxT{b}") for b in range(B)]

            def fill_cd_cols(b):
                # v ones-columns: cd * mask, broadcast over heads
                nc.gpsimd.tensor_copy(
                    out=v_sb[b][:, :, :, 64:65].squeeze(3),
                    in_=maskcd[:, b * NKT:(b + 1) * NKT]
                        .unsqueeze(2).broadcast_to([128, NKT, HPC]),
                )

            # ---- phase 1: QKV projections (fp8 DR, 3-term) ----
            def proj_matmuls(ps, xhi_t, xlo_t, w_pair, w_hi, n0, n1,
                             w_is_lhs):
                """12 DR matmuls accumulating x @ w into ps[128, n1-n0]."""
                first = True
                for kt in range(KT8):
                    if w_is_lhs:
                        nc.tensor.matmul(
                            ps, lhsT=w_pair[:, kt, :, :],
                            rhs=xhi_t[:, kt, n0:n1].unsqueeze(1)
                                .broadcast_to([128, 2, n1 - n0]),
                            start=first, stop=False, perf_mode=DR)
                    else:
                        nc.tensor.matmul(
                            ps, lhsT=xhi_t[:, kt, n0:n1].unsqueeze(1)
                                .broadcast_to([128, 2, n1 - n0]),
                            rhs=w_pair[:, kt, :, :],
                            start=first, stop=False, perf_mode=DR)
                    first = False
                for tp in range(KT8 // 2):
                    last = tp == KT8 // 2 - 1
                    if w_is_lhs:
                        nc.tensor.matmul(
                            ps, lhsT=w_hi[:, 2 * tp:2 * tp + 2, :],
                            rhs=xlo_t[:, 2 * tp:2 * tp + 2, n0:n1],
                            start=False, stop=last, perf_mode=DR)
                    else:
                        nc.tensor.matmul(
                            ps, lhsT=xlo_t[:, 2 * tp:2 * tp + 2, n0:n1],
                            rhs=w_hi[:, 2 * tp:2 * tp + 2, :],
                            start=False, stop=last, perf_mode=DR)

            def phase1_dma(b, chb):
                roff = b * RB + chb * 512
                xhi_t = xt_pool.tile([128, KT8, 512], F8, tag="xhi")
                xlo_t = xt_pool.tile([128, KT8, 512], F8, tag="xlo")
                nc.sync.dma_start(out=xhi_t, in_=xhi_d[:, :, roff:roff + 512])
                nc.sync.dma_start(out=xlo_t, in_=xlo_d[:, :, roff:roff + 512])
                return xhi_t, xlo_t

            def phase1_chunk(b, chb, pre=None, defer_v=False):
                cols = slice(chb * 512, (chb + 1) * 512)
                xhi_t, xlo_t = pre if pre is not None else phase1_dma(b, chb)
                # Q
                qk_eng = nc.vector if QK_ENG == "v" else nc.scalar
                ps = psmm_pool.tile([128, 512], F32, tag="mm", name="qps")
                proj_matmuls(ps, xhi_t, xlo_t, wqp, wqh, 0, 512, True)
                qk_eng.tensor_scalar_add(
                    out=qt[b][:, cols], in0=ps, scalar1=bq_sb)
                # K -> hi/lo pair
                ps = psmm_pool.tile([128, 512], F32, tag="mm", name="kps")
                proj_matmuls(ps, xhi_t, xlo_t, wkp, wkh, 0, 512, True)
                qk_eng.tensor_scalar_add(
                    out=kt2[b][:, 0, cols], in0=ps, scalar1=bk_sb)
                qk_eng.scalar_tensor_tensor(
                    out=kt2[b][:, 1, cols], in0=ps, scalar=bk_sb,
                    in1=kt2[b][:, 0, cols], op0=AluOpType.add,
                    op1=AluOpType.subtract)
                # V (natural): out rows = x columns -> 4 sub-tiles of 128
                def emit_v():
                  for sub in range(4):
                    rt = chb * 4 + sub
                    n0 = sub * 128
                    vps = psmm_pool.tile([128, 128], F32, tag="mm",
                                         name=f"vps{b}{chb}{sub}")
                    proj_matmuls(vps, xhi_t, xlo_t, wvp, wvh, n0, n0 + 128,
                                 False)
                    mcol = maskv[:, b * NKT + rt:b * NKT + rt + 1]
                    for h in range(HPC):
                        vd = v_sb[b][:, rt, h, 0:64]
                        nc.vector.tensor_add(
                            out=vd, in0=vps[:, h * 64:(h + 1) * 64],
                            in1=bv_rep[:, h * 64:(h + 1) * 64])
                        nc.gpsimd.tensor_scalar_mul(
                            out=vd, in0=vd, scalar1=mcol)
                if defer_v:
                    return emit_v
                emit_v()
                return None

            # ---- phase 2: attention ----
            def attn_alloc():
                # ctx accumulators: [128, 4, 128] x2 (h), slot j = qsub
                return [psctx_pool.tile([128, 4, 128], F32, tag="ctx",
                                        name=f"ctxps{h}") for h in range(HPC)]

            def attn_ctx(b, cps, awt, kt):
                # one start per PSUM tile: start zeroing covers the whole
                # tile, so only (kt0, j0) starts; other regions accumulate
                # onto the zeroed tile
                for h in range(HPC):
                    for j in range(4):
                        nc.tensor.matmul(
                            cps[h][:, j, 0:65],
                            lhsT=awt[:, kt, h, j * 128:(j + 1) * 128],
                            rhs=v_sb[b][:, kt, h, 0:65],
                            start=(kt == 0 and j == 0),
                            stop=(kt == NKT - 1),
                            skip_group_check=True)

            def attn_kts(b, qc, cps, kts, pend):
                qs = qc * 512
                for kt in kts:
                    ks = kt * 128
                    sc = pssc_pool.tile([128, HPC, 512], F32, tag="sc",
                                        name="scps")
                    for h in range(HPC):
                        p0 = h * 64
                        nc.tensor.matmul(
                            sc[:, h, :],
                            lhsT=kt2[b][p0:p0 + 64, :, ks:ks + 128],
                            rhs=qt[b][p0:p0 + 64, qs:qs + 512].unsqueeze(1)
                                .broadcast_to([64, 2, 512]),
                            start=True, stop=True, perf_mode=DR,
                            tile_position=(p0, 0))
                    awt = aw_pool.tile([128, NKT, HPC, 512], BF16, tag="aw",
                                       name="awt") if kt == 0 else attn_kts.awt
                    attn_kts.awt = awt
                    if kt in DVE_EXP_KTS:
                        nc.vector._custom_dve(
                            EXP_OP, out=awt[:, kt, :, :], in0=sc,
                            s0=EXP_SCALE / 16.0)
                    else:
                        nc.scalar.activation(
                            out=awt[:, kt, :, :], in_=sc,
                            func=mybir.ActivationFunctionType.Exp,
                            scale=EXP_SCALE)
                    # software pipeline: run the PREVIOUS kt's ctx now, so the
                    # PE never blocks on this kt's exp round-trip
                    while len(pend) >= 2:
                        attn_ctx(b, cps, awt, pend.pop(0))
                    pend.append(kt)

            def attn_finish(b, qc, cps):
                ctxf = small.tile([128, HPC, 4, 65], F32, tag="ctxf",
                                  name="ctxf")
                ctxf_eng = nc.vector if CTXF_ENG == "v" else nc.gpsimd
                for h in range(HPC):
                    ctxf_eng.tensor_copy(
                        out=ctxf[:, h, :, :], in_=cps[h][:, :, 0:65])
                ctxn = small.tile([128, 4, HPC, 64], BF16, tag="ctxn",
                                  name="ctxn")
                for h in range(HPC):
                    for j in range(4):
                        nc.gpsimd.normalize_recip(
                            ctxn[:, j, h, :], ctxf[:, h, j, 0:64],
                            ctxf[:, h, j, 64:65])
                for j in range(4):
                    qs = b * 0 + qc * 512 + j * 128
                    nc.sync.dma_start_transpose(
                        out=ctxT[b][:, qs:qs + 128],
                        in_=ctxn[:, j, :, :])

            # ---- phase 3 ----
            # copy engines: nc.vector / nc.gpsimd / nc.scalar mix per ot
            p3_osb = {}

            def phase3_half(b, rc, half, engs=None):
                engs = engs or (P3_ENGS[:4] if half == 0 else P3_ENGS[4:])
                emap = {"v": nc.vector, "g": nc.gpsimd, "s": nc.scalar}
                if half == 0:
                    p3_osb[(b, rc)] = outsb_pool.tile(
                        [128, OT, 512], BF16, tag="osb", name=f"osb{b}{rc}")
                osb = p3_osb[(b, rc)]
                for i in range(4):
                    ot = half * 4 + i
                    ps = psmm_pool.tile([128, 512], F32, tag="mm",
                                        name="outps")
                    nc.tensor.matmul(
                        ps, lhsT=wo_sb[:, ot * 128:(ot + 1) * 128],
                        rhs=ctxT[b][:, rc * 512:(rc + 1) * 512],
                        start=True, stop=True)
                    eng = emap[engs[i]]
                    if eng is nc.scalar:
                        eng.copy(out=osb[:, ot, :], in_=ps)
                    else:
                        eng.tensor_copy(out=osb[:, ot, :], in_=ps)
                if half == 1:
                    cs = b * RB + rc * 512
                    outT_r = outT_d.rearrange("(t p) r -> p t r", p=128)
                    nc.sync.dma_start(out=outT_r[:, :, cs:cs + 512], in_=osb)

            def phase3(b, rc, engs=None):
                engs = engs or P3_ENGS
                phase3_half(b, rc, 0, engs[:4])
                phase3_half(b, rc, 1, engs[4:])

            # ---- schedule ----
            def attn(b, qc, hooks=None):
                cps = attn_alloc()
                pend = []
                lo = 0
                for at in sorted(hooks or {}):
                    attn_kts(b, qc, cps, list(range(lo, at)), pend)
                    hooks[at]()
                    lo = at
                attn_kts(b, qc, cps, list(range(lo, NKT)), pend)
                while pend:
                    attn_ctx(b, cps, attn_kts.awt, pend.pop(0))
                attn_finish(b, qc, cps)

            pre00 = phase1_dma(0, 0)
            load_consts()
            fill_cd_cols(0)
            fill_cd_cols(1)
            v00 = phase1_chunk(0, 0, pre=pre00, defer_v=True)
            attn(0, 0, {1: v00,
                        4: lambda: phase1_chunk(0, 1),
                        8: lambda: phase1_chunk(0, 2),
                        12: lambda: phase1_chunk(0, 3)})
            attn(0, 1, {4: lambda: phase1_chunk(1, 0),
                        12: lambda: phase1_chunk(1, 1)})
            attn(0, 2, {4: lambda: phase1_chunk(1, 2),
                        12: lambda: phase3_half(0, 0, 0)})
            attn(0, 3, {4: lambda: phase3_half(0, 0, 1),
                        8: lambda: phase1_chunk(1, 3),
                        12: lambda: phase3_half(0, 1, 0)})
            attn(1, 0, {4: lambda: phase3_half(0, 1, 1),
                        12: lambda: phase3_half(0, 2, 0)})
            attn(1, 1, {4: lambda: phase3_half(0, 2, 1),
                        12: lambda: phase3_half(0, 3, 0)})
            attn(1, 2, {4: lambda: phase3_half(0, 3, 1),
                        12: lambda: phase3_half(1, 0, 0)})
            attn(1, 3, {4: lambda: phase3_half(1, 0, 1),
                        8: lambda: phase3_half(1, 1, 0),
                        12: lambda: phase3_half(1, 1, 1)})
            phase3(1, 2, engs="svsvsvsv")
            phase3(1, 3, engs="svsvsvsv")

    nc.compile()
    return nc


@functools.lru_cache(maxsize=1)
def _get_nc():
    return _build_nc()


def _split8(a):
    f8 = ml_dtypes.float8_e4m3
    hi = a.astype(f8)
    lo = (a - hi.astype(np.float32)).astype(f8)
    return hi, lo


def _make_in_maps(x, attention_mask, W_qkv, b_qkv, W_out, b_out):
    bf16 = ml_dtypes.bfloat16
    x = np.asarray(x, dtype=np.float32).reshape(R, D)
    xT = np.ascontiguousarray(x.T)                     # [D, R]
    xk = np.ascontiguousarray(
        xT.reshape(KT8, 128, R).transpose(1, 0, 2))    # [128, KT8, R]
    x_hi, x_lo = _split8(xk)
    W_qkv = np.asarray(W_qkv, dtype=np.float32)
    W_out = np.asarray(W_out, dtype=np.float32)
    b_qkv = np.asarray(b_qkv, dtype=np.float32)
    mask = np.asarray(attention_mask).astype(np.float32)
    maskv = np.ascontiguousarray(
        mask.reshape(B, NKT, 128).transpose(2, 0, 1).reshape(128, B * NKT))
    maskcd = np.ascontiguousarray(maskv * CD)

    def _wsplit(w):  # [1024, 128] -> pair [128, KT8, 2, 128] + hi [128, KT8, 128]
        wk = np.ascontiguousarray(
            (w * SW).reshape(KT8, 128, 128).transpose(1, 0, 2))
        hi, lo = _split8(wk)
        pair = np.ascontiguousarray(
            np.stack([hi, lo], axis=2))                # [128, KT8, 2, 128]
        return pair, np.ascontiguousarray(hi)

    in_maps = []
    for c in range(CORES):
        s = slice(128 * c, 128 * (c + 1))
        wqp, wqh = _wsplit(W_qkv[:, s])
        wkp, wkh = _wsplit(W_qkv[:, D:][:, s])
        wvp, wvh = _wsplit(W_qkv[:, 2 * D:][:, s])
        in_maps.append({
            "xhi": x_hi, "xlo": x_lo,
            "wqp": wqp, "wkp": wkp, "wvp": wvp,
            "wqh": wqh, "wkh": wkh, "wvh": wvh,
            "wo": np.ascontiguousarray(W_out[s, :] * SW).astype(bf16),
            "bqkv": np.ascontiguousarray(
                np.stack([b_qkv[s], b_qkv[D:][s], b_qkv[2 * D:][s]]) * SW),
            "maskv": maskv,
            "maskcd": maskcd,
        })
    return in_maps


def timeline_estimate_ns():
    """Cost-model makespan of the per-core program (no HW needed)."""
    from concourse.timeline_sim import TimelineSim
    return TimelineSim(_get_nc(), trace=False).simulate()


def run(trace=False, **inputs):
    nc = _get_nc()
    b_out = np.asarray(inputs["b_out"], dtype=np.float32)
    in_maps = _make_in_maps(**inputs)
    try:
        res = run_bass_kernel_spmd(
            nc, in_maps, core_ids=list(range(CORES)), trace=trace,
        )
    except (ImportError, ModuleNotFoundError):
        res = run_bass_kernel_spmd(
            nc, in_maps, core_ids=list(range(CORES)), trace=False,
        )
    acc = np.zeros((D, R), dtype=np.float32)
    for r in res.results:
        acc += r["outT"].astype(np.float32)
    out = (np.ascontiguousarray(acc.T) / OUT_SCALE + b_out).reshape(B, N, D)
    return out, res


def kernel(**inputs):
    out, _ = run(trace=False, **inputs)
    return out


# revision 27
# speedup vs baseline: 1.1663x; 1.0206x over previous
"""Multi-head attention (B=2, N=2048, D=1024, H=16) on 8 trn2 NeuronCores.

Sharding: tensor-parallel over heads - core c owns heads (2c, 2c+1); host sums
the 8 full-size partial outputs (row-sharded W_out product) and transposes.

Device dataflow per core (fp8 DoubleRow matmuls where precision allows):
  phase 1 (QKV projections, 0.75 cyc/row): weights are host-split into
    (hi, lo) fp8e4 pairs; x is host-split into x_hi/x_lo fp8e4.
    Per k-tile: one DR matmul (w_hi,w_lo)x(x_hi bcast) + per k-tile-pair one
    DR matmul (w_hi,w_hi)x(x_lo pair) -> full-term fp32 PSUM (drops only the
    O(2^-8) lo*lo term, bf16-class accuracy).
    q -> fp8 (one-sided), k -> (hi,lo) fp8 pair, v -> bf16 natural [keys,dims]
    with a cd*mask denominator column appended.
  phase 2 (attention): scoresT = (k_hi,k_lo) DR-matmul (q,q) at 0.5 cyc/row
    (k at full precision, q one-sided fp8).  exp on ACT engine (bf16 out),
    a fraction of k-tiles on DVE via a custom 8-stage squaring op
    ((1+t/8+t^2/128)^8).  ctx natural orientation: out[queries,65] bf16
    matmuls filling all 128 output partitions (65 cyc/instr); denominators
    ride along as column 64 (cd*mask); gpsimd normalize_recip; DMA-transpose
    [128q, 2hx64d] -> ctxT.
  phase 3: outT_partial = wo^T ctxT (bf16), partials DMA'd as bf16; host sums.
"""

import os
import sys
import functools

import numpy as np
import ml_dtypes

for _p in (
    "/root/.axon_site",
    "/root/.axon_site/_ro/trn_rl_repo",
    "/root/.axon_site/_ro/pypackages",
    "/opt/trn_rl_repo",
):
    if os.path.isdir(_p) and _p not in sys.path:
        sys.path.append(_p)

def _ensure_axon():
    """Register the axon PJRT backend if sitecustomize didn't run."""
    import jax
    try:
        backends = jax._src.xla_bridge._backend_factories
        if "axon" in backends:
            return
    except Exception:
        pass
    try:
        from trn_agent_boot.trn_boot import boot
        os.environ.setdefault("AXON_POOL_SVC_OVERRIDE", "127.0.0.1")
        os.environ.setdefault("AXON_LOOPBACK_RELAY", "1")
        boot(os.environ["TRN_TERMINAL_PRECOMPUTED_JSON"],
             "/opt/axon/libaxon_pjrt.so")
    except Exception:
        pass


_ensure_axon()

import concourse.tile as tile
from concourse import bacc, mybir
from concourse.alu_op_type import AluOpType
from concourse.bass_utils import run_bass_kernel_spmd

BF16 = mybir.dt.bfloat16
F8 = mybir.dt.float8e4
F32 = mybir.dt.float32
DR = mybir.MatmulPerfMode.DoubleRow

B, N, D, H = 2, 2048, 1024, 16
DK = D // H            # 64
CORES = 8
HPC = H // CORES       # 2 heads per core
R = B * N              # 4096 rows total
RB = N                 # rows per batch
KT8 = D // 128         # 8 k-tiles over the model dim
QC = RB // 512         # 4 query chunks of 512 per batch
NKT = RB // 128        # 16 key tiles of 128 per batch
OT = D // 128          # 8 output-dim tiles

SW = 32.0              # weight pre-scale before fp8 split
CD = 1.0 / 8.0         # denominator column constant
EXP_SCALE = (DK ** -0.5) / (SW * SW)
OUT_SCALE = SW * SW / CD          # psum partial = OUT_SCALE * true partial
import os as _os
DVE_EXP_KTS = tuple(int(x) for x in _os.environ.get(
    "K_DVE_KTS", "2,6,10,14").split(",") if x != "")
QK_ENG = _os.environ.get("K_QK_ENG", "v")      # q/k psum->sbuf copy engine
CTXF_ENG = _os.environ.get("K_CTXF_ENG", "v")  # ctx psum->sbuf copy engine
P3_ENGS = _os.environ.get("K_P3_ENGS", "svsvsvsv")

# ---------------------------------------------------------------- custom exp
from concourse import dve_ops as _dve_ops
from concourse.dve_spec import AluOp, Bin, Spec, Src0, One, C0, lower as _dve_lower
from concourse.dve_spec import _has_src1
from concourse.dve_uop import DveOpSpec


def _exp_sq3_ref(in0, in1, s0, s1, imm2):
    d = in0.astype(np.float32) * np.float32(s0)
    e = d + np.float32(1.0)
    base = e * e + d * d
    r1 = base * base
    r2 = r1 * r1
    return r2 * r2


def _register_exp_op():
    name = "EXP_SQ3_ANT"
    if name in _dve_ops._SUB_OPCODE_FOR_NAME:
        return next(o for o in _dve_ops.OPS if o.name == name)
    d = Bin(AluOp.MULTIPLY, Src0, C0)
    e = Bin(AluOp.ADD, d, One)
    base = Bin(AluOp.ADD, Bin(AluOp.MULTIPLY, e, e), Bin(AluOp.MULTIPLY, d, d))
    r1 = Bin(AluOp.MULTIPLY, base, base)
    r2 = Bin(AluOp.MULTIPLY, r1, r1)
    r3 = Bin(AluOp.MULTIPLY, r2, r2)
    spec = Spec(body=r3, reference=_exp_sq3_ref)
    _dve_ops._SUB_OPCODE_FOR_NAME[name] = 17
    shas = {}
    for ver in ("v3", "v4"):
        dspec = DveOpSpec(name=name, opcode=17, uops=_dve_lower(spec, ver=ver),
                          rd1_en=_has_src1(spec))
        shas[ver] = dspec.sha(ver)
    op = _dve_ops.DveOp(name, spec, subdim=False, uops_sha=shas)
    _dve_ops.OPS.append(op)
    _dve_ops.CUSTOM_DVE_SPECS[name] = spec
    return op


EXP_OP = _register_exp_op()


# ------------------------------------------------------------------- program
def _build_nc():
    nc = bacc.Bacc(
        "TRN2", target_bir_lowering=False, debug=False, enable_asserts=False,
        num_devices=CORES,
    )

    xhi_d = nc.dram_tensor("xhi", [128, KT8, R], F8, kind="ExternalInput").ap()
    xlo_d = nc.dram_tensor("xlo", [128, KT8, R], F8, kind="ExternalInput").ap()
    # (hi, lo) weight pairs + hi-only tensors, per projection
    wqp_d = nc.dram_tensor("wqp", [128, KT8, 2, 128], F8, kind="ExternalInput").ap()
    wkp_d = nc.dram_tensor("wkp", [128, KT8, 2, 128], F8, kind="ExternalInput").ap()
    wvp_d = nc.dram_tensor("wvp", [128, KT8, 2, 128], F8, kind="ExternalInput").ap()
    wqh_d = nc.dram_tensor("wqh", [128, KT8, 128], F8, kind="ExternalInput").ap()
    wkh_d = nc.dram_tensor("wkh", [128, KT8, 128], F8, kind="ExternalInput").ap()
    wvh_d = nc.dram_tensor("wvh", [128, KT8, 128], F8, kind="ExternalInput").ap()
    wo_d = nc.dram_tensor("wo", [128, D], BF16, kind="ExternalInput").ap()
    bqkv_d = nc.dram_tensor("bqkv", [3, 128], F32, kind="ExternalInput").ap()
    maskv_d = nc.dram_tensor("maskv", [128, B * NKT], F32, kind="ExternalInput").ap()
    maskcd_d = nc.dram_tensor("maskcd", [128, B * NKT], F32, kind="ExternalInput").ap()
    outT_d = nc.dram_tensor("outT", [D, R], BF16, kind="ExternalOutput").ap()

    with tile.TileContext(nc) as tc:
        with (
            tc.tile_pool(name="persist", bufs=1) as persist,
            tc.tile_pool(name="xt", bufs=3) as xt_pool,
            tc.tile_pool(name="aw", bufs=int(_os.environ.get("K_AWB", "2"))) as aw_pool,
            tc.tile_pool(name="small", bufs=4) as small,
            tc.tile_pool(name="outsb", bufs=2) as outsb_pool,
            tc.tile_pool(name="psmm", bufs=2, space="PSUM") as psmm_pool,
            tc.tile_pool(name="pssc", bufs=2, space="PSUM") as pssc_pool,
            tc.tile_pool(name="psctx", bufs=2, space="PSUM") as psctx_pool,
        ):
            # ---- weights / consts ----
            wqp = persist.tile([128, KT8, 2, 128], F8, tag="wqp")
            wkp = persist.tile([128, KT8, 2, 128], F8, tag="wkp")
            wvp = persist.tile([128, KT8, 2, 128], F8, tag="wvp")
            wqh = persist.tile([128, KT8, 128], F8, tag="wqh")
            wkh = persist.tile([128, KT8, 128], F8, tag="wkh")
            wvh = persist.tile([128, KT8, 128], F8, tag="wvh")
            wo_sb = persist.tile([128, D], BF16, tag="wo")
            bq_sb = persist.tile([128, 1], F32, tag="bq")
            bk_sb = persist.tile([128, 1], F32, tag="bk")
            bv_rep = persist.tile([128, 128], F32, tag="bv")
            maskv = persist.tile([128, B * NKT], F32, tag="maskv")
            maskcd = persist.tile([128, B * NKT], F32, tag="maskcd")

            def load_consts():
                # order: first-needed first (wqp/wqh feed the very first matmul)
                for w_sb, w_d in ((wqp, wqp_d), (wqh, wqh_d),
                                  (wkp, wkp_d), (wkh, wkh_d),
                                  (wvp, wvp_d), (wvh, wvh_d),
                                  (wo_sb, wo_d)):
                    nc.sync.dma_start(out=w_sb, in_=w_d)
                nc.sync.dma_start(
                    out=bq_sb, in_=bqkv_d[0:1, :].rearrange("o p -> p o"))
                nc.sync.dma_start(
                    out=bk_sb, in_=bqkv_d[1:2, :].rearrange("o p -> p o"))
                nc.sync.dma_start(
                    out=bv_rep, in_=bqkv_d[2:3, :].partition_broadcast(128))
                nc.sync.dma_start(out=maskv, in_=maskv_d)
                nc.sync.dma_start(out=maskcd, in_=maskcd_d)

            # ---- persistent activations ----
            qt = [persist.tile([128, RB], F8, tag=f"qt{b}", name=f"qt{b}")
                  for b in range(B)]
            kt2 = [persist.tile([128, 2, RB], F8, tag=f"kt{b}", name=f"kt{b}")
                   for b in range(B)]
            # V natural [keys, kt, h, 66]: cols 0-63 v, col 64 cd*mask
            v_sb = [persist.tile([128, NKT, HPC, 66], BF16, tag=f"v{b}",
                                 name=f"v{b}") for b in range(B)]
            ctxT = [persist.tile([128, RB], BF16, tag=f"ctxT{b}",
                                 name=f"ctxT{b}") for b in range(B)]

            def fill_cd_cols(b):
                # v ones-columns: cd * mask, broadcast over heads
                nc.gpsimd.tensor_copy(
                    out=v_sb[b][:, :, :, 64:65].squeeze(3),
                    in_=maskcd[:, b * NKT:(b + 1) * NKT]
                        .unsqueeze(2).broadcast_to([128, NKT, HPC]),
                )

            # ---- phase 1: QKV projections (fp8 DR, 3-term) ----
            def proj_matmuls(ps, xhi_t, xlo_t, w_pair, w_hi, n0, n1,
                             w_is_lhs):
                """12 DR matmuls accumulating x @ w into ps[128, n1-n0]."""
                first = True
                for kt in range(KT8):
                    if w_is_lhs:
                        nc.tensor.matmul(
                            ps, lhsT=w_pair[:, kt, :, :],
                            rhs=xhi_t[:, kt, n0:n1].unsqueeze(1)
                                .broadcast_to([128, 2, n1 - n0]),
                            start=first, stop=False, perf_mode=DR)
                    else:
                        nc.tensor.matmul(
                            ps, lhsT=xhi_t[:, kt, n0:n1].unsqueeze(1)
                                .broadcast_to([128, 2, n1 - n0]),
                            rhs=w_pair[:, kt, :, :],
                            start=first, stop=False, perf_mode=DR)
                    first = False
                for tp in range(KT8 // 2):
                    last = tp == KT8 // 2 - 1
                    if w_is_lhs:
                        nc.tensor.matmul(
                            ps, lhsT=w_hi[:, 2 * tp:2 * tp + 2, :],
                            rhs=xlo_t[:, 2 * tp:2 * tp + 2, n0:n1],
                            start=False, stop=last, perf_mode=DR)
                    else:
                        nc.tensor.matmul(
                            ps, lhsT=xlo_t[:, 2 * tp:2 * tp + 2, n0:n1],
                            rhs=w_hi[:, 2 * tp:2 * tp + 2, :],
                            start=False, stop=last, perf_mode=DR)

            def phase1_dma(b, chb):
                roff = b * RB + chb * 512
                xhi_t = xt_pool.tile([128, KT8, 512], F8, tag="xhi")
                xlo_t = xt_pool.tile([128, KT8, 512], F8, tag="xlo")
                nc.sync.dma_start(out=xhi_t, in_=xhi_d[:, :, roff:roff + 512])
                nc.sync.dma_start(out=xlo_t, in_=xlo_d[:, :, roff:roff + 512])
                return xhi_t, xlo_t

            def phase1_chunk(b, chb, pre=None, defer_v=False):
                cols = slice(chb * 512, (chb + 1) * 512)
                xhi_t, xlo_t = pre if pre is not None else phase1_dma(b, chb)
                # Q
                qk_eng = nc.vector if QK_ENG == "v" else nc.scalar
                ps = psmm_pool.tile([128, 512], F32, tag="mm", name="qps")
                proj_matmuls(ps, xhi_t, xlo_t, wqp, wqh, 0, 512, True)
                qk_eng.tensor_scalar_add(
                    out=qt[b][:, cols], in0=ps, scalar1=bq_sb)
                # K -> hi/lo pair
                ps = psmm_pool.tile([128, 512], F32, tag="mm", name="kps")
                proj_matmuls(ps, xhi_t, xlo_t, wkp, wkh, 0, 512, True)
                qk_eng.tensor_scalar_add(
                    out=kt2[b][:, 0, cols], in0=ps, scalar1=bk_sb)
                qk_eng.scalar_tensor_tensor(
                    out=kt2[b][:, 1, cols], in0=ps, scalar=bk_sb,
                    in1=kt2[b][:, 0, cols], op0=AluOpType.add,
                    op1=AluOpType.subtract)
                # V (natural): out rows = x columns -> 4 sub-tiles of 128
                def emit_v():
                  for sub in range(4):
                    rt = chb * 4 + sub
                    n0 = sub * 128
                    vps = psmm_pool.tile([128, 128], F32, tag="mm",
                                         name=f"vps{b}{chb}{sub}")
                    proj_matmuls(vps, xhi_t, xlo_t, wvp, wvh, n0, n0 + 128,
                                 False)
                    mcol = maskv[:, b * NKT + rt:b * NKT + rt + 1]
                    for h in range(HPC):
                        vd = v_sb[b][:, rt, h, 0:64]
                        nc.vector.tensor_add(
                            out=vd, in0=vps[:, h * 64:(h + 1) * 64],
                            in1=bv_rep[:, h * 64:(h + 1) * 64])
                        nc.gpsimd.tensor_scalar_mul(
                            out=vd, in0=vd, scalar1=mcol)
                if defer_v:
                    return emit_v
                emit_v()
                return None

            # ---- phase 2: attention ----
            def attn_alloc():
                # ctx accumulators: [128, 4, 128] x2 (h), slot j = qsub
                return [psctx_pool.tile([128, 4, 128], F32, tag="ctx",
                                        name=f"ctxps{h}") for h in range(HPC)]

            def attn_ctx(b, cps, awt, kt):
                # one start per PSUM tile: start zeroing covers the whole
                # tile, so only (kt0, j0) starts; other regions accumulate
                # onto the zeroed tile
                for h in range(HPC):
                    for j in range(4):
                        nc.tensor.matmul(
                            cps[h][:, j, 0:65],
                            lhsT=awt[:, kt, h, j * 128:(j + 1) * 128],
                            rhs=v_sb[b][:, kt, h, 0:65],
                            start=(kt == 0 and j == 0),
                            stop=(kt == NKT - 1),
                            skip_group_check=True)

            def attn_kts(b, qc, cps, kts, pend):
                qs = qc * 512
                for kt in kts:
                    ks = kt * 128
                    sc = pssc_pool.tile([128, HPC, 512], F32, tag="sc",
                                        name="scps")
                    for h in range(HPC):
                        p0 = h * 64
                        nc.tensor.matmul(
                            sc[:, h, :],
                            lhsT=kt2[b][p0:p0 + 64, :, ks:ks + 128],
                            rhs=qt[b][p0:p0 + 64, qs:qs + 512].unsqueeze(1)
                                .broadcast_to([64, 2, 512]),
                            start=True, stop=True, perf_mode=DR,
                            tile_position=(p0, 0))
                    awt = aw_pool.tile([128, NKT, HPC, 512], BF16, tag="aw",
                                       name="awt") if kt == 0 else attn_kts.awt
                    attn_kts.awt = awt
                    if kt in DVE_EXP_KTS:
                        nc.vector._custom_dve(
                            EXP_OP, out=awt[:, kt, :, :], in0=sc,
                            s0=EXP_SCALE / 16.0)
                    else:
                        nc.scalar.activation(
                            out=awt[:, kt, :, :], in_=sc,
                            func=mybir.ActivationFunctionType.Exp,
                            scale=EXP_SCALE)
                    # software pipeline: run the PREVIOUS kt's ctx now, so the
                    # PE never blocks on this kt's exp round-trip
                    while len(pend) >= int(_os.environ.get('K_PEND', '6')):
                        attn_ctx(b, cps, awt, pend.pop(0))
                    pend.append(kt)

            def attn_finish(b, qc, cps):
                ctxf = small.tile([128, HPC, 4, 65], F32, tag="ctxf",
                                  name="ctxf")
                ctxf_eng = nc.vector if CTXF_ENG == "v" else nc.gpsimd
                for h in range(HPC):
                    ctxf_eng.tensor_copy(
                        out=ctxf[:, h, :, :], in_=cps[h][:, :, 0:65])
                ctxn = small.tile([128, 4, HPC, 64], BF16, tag="ctxn",
                                  name="ctxn")
                for h in range(HPC):
                    for j in range(4):
                        nc.gpsimd.normalize_recip(
                            ctxn[:, j, h, :], ctxf[:, h, j, 0:64],
                            ctxf[:, h, j, 64:65])
                for j in range(4):
                    qs = b * 0 + qc * 512 + j * 128
                    nc.sync.dma_start_transpose(
                        out=ctxT[b][:, qs:qs + 128],
                        in_=ctxn[:, j, :, :])

            # ---- phase 3 ----
            # copy engines: nc.vector / nc.gpsimd / nc.scalar mix per ot
            p3_osb = {}

            def phase3_half(b, rc, half, engs=None):
                engs = engs or (P3_ENGS[:4] if half == 0 else P3_ENGS[4:])
                emap = {"v": nc.vector, "g": nc.gpsimd, "s": nc.scalar}
                if half == 0:
                    p3_osb[(b, rc)] = outsb_pool.tile(
                        [128, OT, 512], BF16, tag="osb", name=f"osb{b}{rc}")
                osb = p3_osb[(b, rc)]
                for i in range(4):
                    ot = half * 4 + i
                    ps = psmm_pool.tile([128, 512], F32, tag="mm",
                                        name="outps")
                    nc.tensor.matmul(
                        ps, lhsT=wo_sb[:, ot * 128:(ot + 1) * 128],
                        rhs=ctxT[b][:, rc * 512:(rc + 1) * 512],
                        start=True, stop=True)
                    eng = emap[engs[i]]
                    if eng is nc.scalar:
                        eng.copy(out=osb[:, ot, :], in_=ps)
                    else:
                        eng.tensor_copy(out=osb[:, ot, :], in_=ps)
                if half == 1:
                    cs = b * RB + rc * 512
                    outT_r = outT_d.rearrange("(t p) r -> p t r", p=128)
                    nc.sync.dma_start(out=outT_r[:, :, cs:cs + 512], in_=osb)

            def phase3(b, rc, engs=None):
                engs = engs or P3_ENGS
                phase3_half(b, rc, 0, engs[:4])
                phase3_half(b, rc, 1, engs[4:])

            # ---- schedule ----
            def attn(b, qc, hooks=None):
                cps = attn_alloc()
                pend = []
                lo = 0
                for at in sorted(hooks or {}):
                    attn_kts(b, qc, cps, list(range(lo, at)), pend)
                    hooks[at]()
                    lo = at
                attn_kts(b, qc, cps, list(range(lo, NKT)), pend)
                while pend:
                    attn_ctx(b, cps, attn_kts.awt, pend.pop(0))
                attn_finish(b, qc, cps)

            pre00 = phase1_dma(0, 0)
            load_consts()
            fill_cd_cols(0)
            fill_cd_cols(1)
            v00 = phase1_chunk(0, 0, pre=pre00, defer_v=True)
            attn(0, 0, {1: v00,
                        4: lambda: phase1_chunk(0, 1),
                        8: lambda: phase1_chunk(0, 2),
                        12: lambda: phase1_chunk(0, 3)})
            attn(0, 1, {4: lambda: phase1_chunk(1, 0),
                        12: lambda: phase1_chunk(1, 1)})
            attn(0, 2, {4: lambda: phase1_chunk(1, 2),
                        12: lambda: phase3_half(0, 0, 0)})
            attn(0, 3, {4: lambda: phase3_half(0, 0, 1),
                        8: lambda: phase1_chunk(1, 3),
                        12: lambda: phase3_half(0, 1, 0)})
            attn(1, 0, {4: lambda: phase3_half(0, 1, 1),
                        12: lambda: phase3_half(0, 2, 0)})
            attn(1, 1, {4: lambda: phase3_half(0, 2, 1),
                        12: lambda: phase3_half(0, 3, 0)})
            attn(1, 2, {4: lambda: phase3_half(0, 3, 1),
                        12: lambda: phase3_half(1, 0, 0)})
            attn(1, 3, {4: lambda: phase3_half(1, 0, 1),
                        8: lambda: phase3_half(1, 1, 0),
                        12: lambda: phase3_half(1, 1, 1)})
            phase3(1, 2, engs="svsvsvsv")
            phase3(1, 3, engs="svsvsvsv")

    nc.compile()
    return nc


@functools.lru_cache(maxsize=1)
def _get_nc():
    return _build_nc()


def _split8(a):
    f8 = ml_dtypes.float8_e4m3
    hi = a.astype(f8)
    lo = (a - hi.astype(np.float32)).astype(f8)
    return hi, lo


def _make_in_maps(x, attention_mask, W_qkv, b_qkv, W_out, b_out):
    bf16 = ml_dtypes.bfloat16
    x = np.asarray(x, dtype=np.float32).reshape(R, D)
    xT = np.ascontiguousarray(x.T)                     # [D, R]
    xk = np.ascontiguousarray(
        xT.reshape(KT8, 128, R).transpose(1, 0, 2))    # [128, KT8, R]
    x_hi, x_lo = _split8(xk)
    W_qkv = np.asarray(W_qkv, dtype=np.float32)
    W_out = np.asarray(W_out, dtype=np.float32)
    b_qkv = np.asarray(b_qkv, dtype=np.float32)
    mask = np.asarray(attention_mask).astype(np.float32)
    maskv = np.ascontiguousarray(
        mask.reshape(B, NKT, 128).transpose(2, 0, 1).reshape(128, B * NKT))
    maskcd = np.ascontiguousarray(maskv * CD)

    def _wsplit(w):  # [1024, 128] -> pair [128, KT8, 2, 128] + hi [128, KT8, 128]
        wk = np.ascontiguousarray(
            (w * SW).reshape(KT8, 128, 128).transpose(1, 0, 2))
        hi, lo = _split8(wk)
        pair = np.ascontiguousarray(
            np.stack([hi, lo], axis=2))                # [128, KT8, 2, 128]
        return pair, np.ascontiguousarray(hi)

    in_maps = []
    for c in range(CORES):
        s = slice(128 * c, 128 * (c + 1))
        wqp, wqh = _wsplit(W_qkv[:, s])
        wkp, wkh = _wsplit(W_qkv[:, D:][:, s])
        wvp, wvh = _wsplit(W_qkv[:, 2 * D:][:, s])
        in_maps.append({
            "xhi": x_hi, "xlo": x_lo,
            "wqp": wqp, "wkp": wkp, "wvp": wvp,
            "wqh": wqh, "wkh": wkh, "wvh": wvh,
            "wo": np.ascontiguousarray(W_out[s, :] * SW).astype(bf16),
            "bqkv": np.ascontiguousarray(
                np.stack([b_qkv[s], b_qkv[D:][s], b_qkv[2 * D:][s]]) * SW),
            "maskv": maskv,
            "maskcd": maskcd,
        })
    return in_maps


def timeline_estimate_ns():
    """Cost-model makespan of the per-core program (no HW needed)."""
    from concourse.timeline_sim import TimelineSim
    return TimelineSim(_get_nc(), trace=False).simulate()


def run(trace=False, **inputs):
    nc = _get_nc()
    b_out = np.asarray(inputs["b_out"], dtype=np.float32)
    in_maps = _make_in_maps(**inputs)
    try:
        res = run_bass_kernel_spmd(
            nc, in_maps, core_ids=list(range(CORES)), trace=trace,
        )
    except (ImportError, ModuleNotFoundError):
        res = run_bass_kernel_spmd(
            nc, in_maps, core_ids=list(range(CORES)), trace=False,
        )
    acc = np.zeros((D, R), dtype=np.float32)
    for r in res.results:
        acc += r["outT"].astype(np.float32)
    out = (np.ascontiguousarray(acc.T) / OUT_SCALE + b_out).reshape(B, N, D)
    return out, res


def kernel(**inputs):
    out, _ = run(trace=False, **inputs)
    return out


# revision 35
# speedup vs baseline: 1.1829x; 1.0143x over previous
"""Multi-head attention (B=2, N=2048, D=1024, H=16) on 8 trn2 NeuronCores.

Sharding: tensor-parallel over heads - core c owns heads (2c, 2c+1); host sums
the 8 full-size partial outputs (row-sharded W_out product) and transposes.

Device dataflow per core (fp8 DoubleRow matmuls where precision allows):
  phase 1 (QKV projections, 0.75 cyc/row): weights are host-split into
    (hi, lo) fp8e4 pairs; x is host-split into x_hi/x_lo fp8e4.
    Per k-tile: one DR matmul (w_hi,w_lo)x(x_hi bcast) + per k-tile-pair one
    DR matmul (w_hi,w_hi)x(x_lo pair) -> full-term fp32 PSUM (drops only the
    O(2^-8) lo*lo term, bf16-class accuracy).
    q -> fp8 (one-sided), k -> (hi,lo) fp8 pair, v -> bf16 natural [keys,dims]
    with a cd*mask denominator column appended.
  phase 2 (attention): scoresT = (k_hi,k_lo) DR-matmul (q,q) at 0.5 cyc/row
    (k at full precision, q one-sided fp8).  exp on ACT engine (bf16 out),
    a fraction of k-tiles on DVE via a custom 8-stage squaring op
    ((1+t/8+t^2/128)^8).  ctx natural orientation: out[queries,65] bf16
    matmuls filling all 128 output partitions (65 cyc/instr); denominators
    ride along as column 64 (cd*mask); gpsimd normalize_recip; DMA-transpose
    [128q, 2hx64d] -> ctxT.
  phase 3: outT_partial = wo^T ctxT (bf16), partials DMA'd as bf16; host sums.
"""

import os
import sys
import functools

import numpy as np
import ml_dtypes

for _p in (
    "/root/.axon_site",
    "/root/.axon_site/_ro/trn_rl_repo",
    "/root/.axon_site/_ro/pypackages",
    "/opt/trn_rl_repo",
):
    if os.path.isdir(_p) and _p not in sys.path:
        sys.path.append(_p)

def _ensure_axon():
    """Register the axon PJRT backend if sitecustomize didn't run."""
    import jax
    try:
        backends = jax._src.xla_bridge._backend_factories
        if "axon" in backends:
            return
    except Exception:
        pass
    try:
        from trn_agent_boot.trn_boot import boot
        os.environ.setdefault("AXON_POOL_SVC_OVERRIDE", "127.0.0.1")
        os.environ.setdefault("AXON_LOOPBACK_RELAY", "1")
        boot(os.environ["TRN_TERMINAL_PRECOMPUTED_JSON"],
             "/opt/axon/libaxon_pjrt.so")
    except Exception:
        pass


_ensure_axon()

import concourse.tile as tile
from concourse import bacc, mybir
from concourse.alu_op_type import AluOpType
from concourse.bass_utils import run_bass_kernel_spmd

BF16 = mybir.dt.bfloat16
F8 = mybir.dt.float8e4
F32 = mybir.dt.float32
DR = mybir.MatmulPerfMode.DoubleRow

B, N, D, H = 2, 2048, 1024, 16
DK = D // H            # 64
CORES = 8
HPC = H // CORES       # 2 heads per core
R = B * N              # 4096 rows total
RB = N                 # rows per batch
KT8 = D // 128         # 8 k-tiles over the model dim
QC = RB // 512         # 4 query chunks of 512 per batch
NKT = RB // 128        # 16 key tiles of 128 per batch
OT = D // 128          # 8 output-dim tiles

SW = 32.0              # weight pre-scale before fp8 split
CD = 1.0 / 8.0         # denominator column constant
EXP_SCALE = (DK ** -0.5) / (SW * SW)
OUT_SCALE = SW * SW / CD          # psum partial = OUT_SCALE * true partial
import os as _os
DVE_EXP_KTS = tuple(int(x) for x in _os.environ.get(
    "K_DVE_KTS", "2,6,10,14").split(",") if x != "")
QK_ENG = _os.environ.get("K_QK_ENG", "v")      # q/k psum->sbuf copy engine
CTXF_ENG = _os.environ.get("K_CTXF_ENG", "v")  # ctx psum->sbuf copy engine
P3_ENGS = _os.environ.get("K_P3_ENGS", "svsvsvsv")

# ---------------------------------------------------------------- custom exp
from concourse import dve_ops as _dve_ops
from concourse.dve_spec import AluOp, Bin, Spec, Src0, One, C0, lower as _dve_lower
from concourse.dve_spec import _has_src1
from concourse.dve_uop import DveOpSpec


def _exp_sq3_ref(in0, in1, s0, s1, imm2):
    d = in0.astype(np.float32) * np.float32(s0)
    e = d + np.float32(1.0)
    base = e * e + d * d
    r1 = base * base
    r2 = r1 * r1
    return r2 * r2


def _register_exp_op():
    name = "EXP_SQ3_ANT"
    if name in _dve_ops._SUB_OPCODE_FOR_NAME:
        return next(o for o in _dve_ops.OPS if o.name == name)
    d = Bin(AluOp.MULTIPLY, Src0, C0)
    e = Bin(AluOp.ADD, d, One)
    base = Bin(AluOp.ADD, Bin(AluOp.MULTIPLY, e, e), Bin(AluOp.MULTIPLY, d, d))
    r1 = Bin(AluOp.MULTIPLY, base, base)
    r2 = Bin(AluOp.MULTIPLY, r1, r1)
    r3 = Bin(AluOp.MULTIPLY, r2, r2)
    spec = Spec(body=r3, reference=_exp_sq3_ref)
    _dve_ops._SUB_OPCODE_FOR_NAME[name] = 17
    shas = {}
    for ver in ("v3", "v4"):
        dspec = DveOpSpec(name=name, opcode=17, uops=_dve_lower(spec, ver=ver),
                          rd1_en=_has_src1(spec))
        shas[ver] = dspec.sha(ver)
    op = _dve_ops.DveOp(name, spec, subdim=False, uops_sha=shas)
    _dve_ops.OPS.append(op)
    _dve_ops.CUSTOM_DVE_SPECS[name] = spec
    return op


EXP_OP = _register_exp_op()


# ------------------------------------------------------------------- program
def _build_nc(ZERO_BIAS=True):
    nc = bacc.Bacc(
        "TRN2", target_bir_lowering=False, debug=False, enable_asserts=False,
        num_devices=CORES,
    )

    xhi_d = nc.dram_tensor("xhi", [128, KT8, R], F8, kind="ExternalInput").ap()
    xlo_d = nc.dram_tensor("xlo", [128, KT8, R], F8, kind="ExternalInput").ap()
    # (hi, lo) weight pairs + hi-only tensors, per projection
    wqp_d = nc.dram_tensor("wqp", [128, KT8, 2, 128], F8, kind="ExternalInput").ap()
    wkp_d = nc.dram_tensor("wkp", [128, KT8, 2, 128], F8, kind="ExternalInput").ap()
    wvp_d = nc.dram_tensor("wvp", [128, KT8, 2, 128], F8, kind="ExternalInput").ap()
    wqh_d = nc.dram_tensor("wqh", [128, KT8, 128], F8, kind="ExternalInput").ap()
    wkh_d = nc.dram_tensor("wkh", [128, KT8, 128], F8, kind="ExternalInput").ap()
    wvh_d = nc.dram_tensor("wvh", [128, KT8, 128], F8, kind="ExternalInput").ap()
    wo_d = nc.dram_tensor("wo", [128, D], BF16, kind="ExternalInput").ap()
    bqkv_d = nc.dram_tensor("bqkv", [3, 128], F32, kind="ExternalInput").ap()
    maskv_d = nc.dram_tensor("maskv", [128, B * NKT], F32, kind="ExternalInput").ap()
    maskcd_d = nc.dram_tensor("maskcd", [128, B * NKT], F32, kind="ExternalInput").ap()
    outT_d = nc.dram_tensor("outT", [D, R], BF16, kind="ExternalOutput").ap()

    with tile.TileContext(nc) as tc:
        with (
            tc.tile_pool(name="persist", bufs=1) as persist,
            tc.tile_pool(name="xt", bufs=int(_os.environ.get("K_XTB", "3"))) as xt_pool,
            tc.tile_pool(name="aw", bufs=int(_os.environ.get("K_AWB", "2"))) as aw_pool,
            tc.tile_pool(name="small", bufs=int(_os.environ.get("K_SMB", "4"))) as small,
            tc.tile_pool(name="outsb", bufs=int(_os.environ.get("K_OSB", "2"))) as outsb_pool,
            tc.tile_pool(name="psmm", bufs=2, space="PSUM") as psmm_pool,
            tc.tile_pool(name="pssc", bufs=2, space="PSUM") as pssc_pool,
            tc.tile_pool(name="psctx", bufs=2, space="PSUM") as psctx_pool,
        ):
            # ---- weights / consts ----
            wqp = persist.tile([128, KT8, 2, 128], F8, tag="wqp")
            wkp = persist.tile([128, KT8, 2, 128], F8, tag="wkp")
            wvp = persist.tile([128, KT8, 2, 128], F8, tag="wvp")
            wqh = persist.tile([128, KT8, 128], F8, tag="wqh")
            wkh = persist.tile([128, KT8, 128], F8, tag="wkh")
            wvh = persist.tile([128, KT8, 128], F8, tag="wvh")
            wo_sb = persist.tile([128, D], BF16, tag="wo")
            bq_sb = persist.tile([128, 1], F32, tag="bq")
            bk_sb = persist.tile([128, 1], F32, tag="bk")
            bv_rep = persist.tile([128, 128], F32, tag="bv")
            maskv = persist.tile([128, B * NKT], F32, tag="maskv")
            maskcd = persist.tile([128, B * NKT], F32, tag="maskcd")

            def load_consts():
                # order: first-needed first (wqp/wqh feed the very first matmul)
                for w_sb, w_d in ((wqp, wqp_d), (wqh, wqh_d),
                                  (wkp, wkp_d), (wkh, wkh_d),
                                  (wvp, wvp_d), (wvh, wvh_d),
                                  (wo_sb, wo_d)):
                    nc.sync.dma_start(out=w_sb, in_=w_d)
                nc.sync.dma_start(
                    out=bq_sb, in_=bqkv_d[0:1, :].rearrange("o p -> p o"))
                nc.sync.dma_start(
                    out=bk_sb, in_=bqkv_d[1:2, :].rearrange("o p -> p o"))
                nc.sync.dma_start(
                    out=bv_rep, in_=bqkv_d[2:3, :].partition_broadcast(128))
                nc.sync.dma_start(out=maskv, in_=maskv_d)
                nc.sync.dma_start(out=maskcd, in_=maskcd_d)

            # ---- persistent activations ----
            qt = [persist.tile([128, RB], F8, tag=f"qt{b}", name=f"qt{b}")
                  for b in range(B)]
            kt2 = [persist.tile([128, 2, RB], F8, tag=f"kt{b}", name=f"kt{b}")
                   for b in range(B)]
            # V natural [keys, kt, h, 66]: cols 0-63 v, col 64 cd*mask
            v_sb = [persist.tile([128, NKT, HPC, 66], BF16, tag=f"v{b}",
                                 name=f"v{b}") for b in range(B)]
            ctxT = [persist.tile([128, RB], BF16, tag=f"ctxT{b}",
                                 name=f"ctxT{b}") for b in range(B)]

            def fill_cd_cols(b):
                # v ones-columns: cd * mask, broadcast over heads
                nc.gpsimd.tensor_copy(
                    out=v_sb[b][:, :, :, 64:65].squeeze(3),
                    in_=maskcd[:, b * NKT:(b + 1) * NKT]
                        .unsqueeze(2).broadcast_to([128, NKT, HPC]),
                )

            # ---- phase 1: QKV projections (fp8 DR, 3-term) ----
            def proj_matmuls(ps, xhi_t, xlo_t, w_pair, w_hi, n0, n1,
                             w_is_lhs):
                """12 DR matmuls accumulating x @ w into ps[128, n1-n0]."""
                first = True
                for kt in range(KT8):
                    if w_is_lhs:
                        nc.tensor.matmul(
                            ps, lhsT=w_pair[:, kt, :, :],
                            rhs=xhi_t[:, kt, n0:n1].unsqueeze(1)
                                .broadcast_to([128, 2, n1 - n0]),
                            start=first, stop=False, perf_mode=DR)
                    else:
                        nc.tensor.matmul(
                            ps, lhsT=xhi_t[:, kt, n0:n1].unsqueeze(1)
                                .broadcast_to([128, 2, n1 - n0]),
                            rhs=w_pair[:, kt, :, :],
                            start=first, stop=False, perf_mode=DR)
                    first = False
                for tp in range(KT8 // 2):
                    last = tp == KT8 // 2 - 1
                    if w_is_lhs:
                        nc.tensor.matmul(
                            ps, lhsT=w_hi[:, 2 * tp:2 * tp + 2, :],
                            rhs=xlo_t[:, 2 * tp:2 * tp + 2, n0:n1],
                            start=False, stop=last, perf_mode=DR)
                    else:
                        nc.tensor.matmul(
                            ps, lhsT=xlo_t[:, 2 * tp:2 * tp + 2, n0:n1],
                            rhs=w_hi[:, 2 * tp:2 * tp + 2, :],
                            start=False, stop=last, perf_mode=DR)

            def phase1_dma(b, chb):
                roff = b * RB + chb * 512
                xhi_t = xt_pool.tile([128, KT8, 512], F8, tag="xhi")
                xlo_t = xt_pool.tile([128, KT8, 512], F8, tag="xlo")
                nc.sync.dma_start(out=xhi_t, in_=xhi_d[:, :, roff:roff + 512])
                nc.sync.dma_start(out=xlo_t, in_=xlo_d[:, :, roff:roff + 512])
                return xhi_t, xlo_t

            def phase1_chunk(b, chb, pre=None, defer_v=False):
                cols = slice(chb * 512, (chb + 1) * 512)
                xhi_t, xlo_t = pre if pre is not None else phase1_dma(b, chb)
                # Q
                qk_eng = nc.vector if QK_ENG == "v" else nc.scalar
                ps = psmm_pool.tile([128, 512], F32, tag="mm", name="qps")
                proj_matmuls(ps, xhi_t, xlo_t, wqp, wqh, 0, 512, True)
                qk_eng.tensor_scalar_add(
                    out=qt[b][:, cols], in0=ps, scalar1=bq_sb)
                # K -> hi/lo pair
                ps = psmm_pool.tile([128, 512], F32, tag="mm", name="kps")
                proj_matmuls(ps, xhi_t, xlo_t, wkp, wkh, 0, 512, True)
                qk_eng.tensor_scalar_add(
                    out=kt2[b][:, 0, cols], in0=ps, scalar1=bk_sb)
                qk_eng.scalar_tensor_tensor(
                    out=kt2[b][:, 1, cols], in0=ps, scalar=bk_sb,
                    in1=kt2[b][:, 0, cols], op0=AluOpType.add,
                    op1=AluOpType.subtract)
                # V (natural): out rows = x columns -> 4 sub-tiles of 128
                def emit_v():
                  for sub in range(4):
                    rt = chb * 4 + sub
                    n0 = sub * 128
                    vps = psmm_pool.tile([128, 128], F32, tag="mm",
                                         name=f"vps{b}{chb}{sub}")
                    proj_matmuls(vps, xhi_t, xlo_t, wvp, wvh, n0, n0 + 128,
                                 False)
                    mcol = maskv[:, b * NKT + rt:b * NKT + rt + 1]
                    for h in range(HPC):
                        vd = v_sb[b][:, rt, h, 0:64]
                        if ZERO_BIAS:
                            nc.vector.tensor_scalar_mul(
                                out=vd, in0=vps[:, h * 64:(h + 1) * 64],
                                scalar1=mcol)
                        else:
                            nc.vector.tensor_add(
                                out=vd, in0=vps[:, h * 64:(h + 1) * 64],
                                in1=bv_rep[:, h * 64:(h + 1) * 64])
                            nc.gpsimd.tensor_scalar_mul(
                                out=vd, in0=vd, scalar1=mcol)
                if defer_v:
                    return emit_v
                emit_v()
                return None

            # ---- phase 2: attention ----
            def attn_alloc():
                # ctx accumulators: [128, 4, 128] x2 (h), slot j = qsub
                return [psctx_pool.tile([128, 4, 128], F32, tag="ctx",
                                        name=f"ctxps{h}") for h in range(HPC)]

            def attn_ctx(b, cps, awt, kt):
                # one start per PSUM tile: start zeroing covers the whole
                # tile, so only (kt0, j0) starts; other regions accumulate
                # onto the zeroed tile
                for h in range(HPC):
                    for j in range(4):
                        nc.tensor.matmul(
                            cps[h][:, j, 0:65],
                            lhsT=awt[:, kt, h, j * 128:(j + 1) * 128],
                            rhs=v_sb[b][:, kt, h, 0:65],
                            start=(kt == 0 and j == 0),
                            stop=(kt == NKT - 1),
                            skip_group_check=True)

            def attn_kts(b, qc, cps, kts, pend):
                qs = qc * 512
                for kt in kts:
                    ks = kt * 128
                    sc = pssc_pool.tile([128, HPC, 512], F32, tag="sc",
                                        name="scps")
                    for h in range(HPC):
                        p0 = h * 64
                        nc.tensor.matmul(
                            sc[:, h, :],
                            lhsT=kt2[b][p0:p0 + 64, :, ks:ks + 128],
                            rhs=qt[b][p0:p0 + 64, qs:qs + 512].unsqueeze(1)
                                .broadcast_to([64, 2, 512]),
                            start=True, stop=True, perf_mode=DR,
                            tile_position=(p0, 0))
                    awt = aw_pool.tile([128, NKT, HPC, 512], BF16, tag="aw",
                                       name="awt") if kt == 0 else attn_kts.awt
                    attn_kts.awt = awt
                    if kt in DVE_EXP_KTS:
                        nc.vector._custom_dve(
                            EXP_OP, out=awt[:, kt, :, :], in0=sc,
                            s0=EXP_SCALE / 16.0)
                    else:
                        nc.scalar.activation(
                            out=awt[:, kt, :, :], in_=sc,
                            func=mybir.ActivationFunctionType.Exp,
                            scale=EXP_SCALE)
                    # software pipeline: run the PREVIOUS kt's ctx now, so the
                    # PE never blocks on this kt's exp round-trip
                    while len(pend) >= int(_os.environ.get('K_PEND', '6')):
                        attn_ctx(b, cps, awt, pend.pop(0))
                    pend.append(kt)

            def attn_finish(b, qc, cps):
                rcp = small.tile([128, HPC, 4], F32, tag="rcp", name="rcp")
                ctxn = small.tile([128, 4, HPC, 64], BF16, tag="ctxn",
                                  name="ctxn")
                for h in range(HPC):
                    nc.vector.reciprocal(
                        rcp[:, h, :], cps[h][:, :, 64:65].squeeze(2))
                    nc.vector.tensor_mul(
                        out=ctxn[:, :, h, :], in0=cps[h][:, :, 0:64],
                        in1=rcp[:, h, :].unsqueeze(2).broadcast_to(
                            [128, 4, 64]))
                for j in range(4):
                    qs = b * 0 + qc * 512 + j * 128
                    nc.sync.dma_start_transpose(
                        out=ctxT[b][:, qs:qs + 128],
                        in_=ctxn[:, j, :, :])

            # ---- phase 3 ----
            # copy engines: nc.vector / nc.gpsimd / nc.scalar mix per ot
            p3_osb = {}

            def phase3_half(b, rc, half, engs=None):
                engs = engs or (P3_ENGS[:4] if half == 0 else P3_ENGS[4:])
                emap = {"v": nc.vector, "g": nc.gpsimd, "s": nc.scalar}
                if half == 0:
                    p3_osb[(b, rc)] = outsb_pool.tile(
                        [128, OT, 512], BF16, tag="osb", name=f"osb{b}{rc}")
                osb = p3_osb[(b, rc)]
                for i in range(4):
                    ot = half * 4 + i
                    ps = psmm_pool.tile([128, 512], F32, tag="mm",
                                        name="outps")
                    nc.tensor.matmul(
                        ps, lhsT=wo_sb[:, ot * 128:(ot + 1) * 128],
                        rhs=ctxT[b][:, rc * 512:(rc + 1) * 512],
                        start=True, stop=True)
                    eng = emap[engs[i]]
                    if eng is nc.scalar:
                        eng.copy(out=osb[:, ot, :], in_=ps)
                    else:
                        eng.tensor_copy(out=osb[:, ot, :], in_=ps)
                if half == 1:
                    cs = b * RB + rc * 512
                    outT_r = outT_d.rearrange("(t p) r -> p t r", p=128)
                    nc.sync.dma_start(out=outT_r[:, :, cs:cs + 512], in_=osb)

            def phase3(b, rc, engs=None):
                engs = engs or P3_ENGS
                phase3_half(b, rc, 0, engs[:4])
                phase3_half(b, rc, 1, engs[4:])

            # ---- schedule ----
            def attn(b, qc, hooks=None):
                cps = attn_alloc()
                pend = []
                lo = 0
                for at in sorted(hooks or {}):
                    attn_kts(b, qc, cps, list(range(lo, at)), pend)
                    hooks[at]()
                    lo = at
                attn_kts(b, qc, cps, list(range(lo, NKT)), pend)
                while pend:
                    attn_ctx(b, cps, attn_kts.awt, pend.pop(0))
                attn_finish(b, qc, cps)

            pre00 = phase1_dma(0, 0)
            load_consts()
            fill_cd_cols(0)
            fill_cd_cols(1)
            v00 = phase1_chunk(0, 0, pre=pre00, defer_v=True)
            attn(0, 0, {1: v00,
                        4: lambda: phase1_chunk(0, 1),
                        8: lambda: phase1_chunk(0, 2),
                        12: lambda: phase1_chunk(0, 3)})
            attn(0, 1, {4: lambda: phase1_chunk(1, 0),
                        12: lambda: phase1_chunk(1, 1)})
            attn(0, 2, {4: lambda: phase1_chunk(1, 2),
                        12: lambda: phase3_half(0, 0, 0)})
            attn(0, 3, {4: lambda: phase3_half(0, 0, 1),
                        8: lambda: phase1_chunk(1, 3),
                        12: lambda: phase3_half(0, 1, 0)})
            attn(1, 0, {4: lambda: phase3_half(0, 1, 1),
                        12: lambda: phase3_half(0, 2, 0)})
            attn(1, 1, {4: lambda: phase3_half(0, 2, 1),
                        12: lambda: phase3_half(0, 3, 0)})
            attn(1, 2, {4: lambda: phase3_half(0, 3, 1),
                        12: lambda: phase3_half(1, 0, 0)})
            attn(1, 3, {4: lambda: phase3_half(1, 0, 1),
                        8: lambda: phase3_half(1, 1, 0),
                        12: lambda: phase3_half(1, 1, 1)})
            phase3(1, 2, engs="svsvsvsv")
            phase3(1, 3, engs="svsvsvsv")

    nc.compile()
    return nc


@functools.lru_cache(maxsize=2)
def _get_nc(zero_bias=True):
    return _build_nc(zero_bias)


def _split8(a):
    f8 = ml_dtypes.float8_e4m3
    hi = a.astype(f8)
    lo = (a - hi.astype(np.float32)).astype(f8)
    return hi, lo


def _make_in_maps(x, attention_mask, W_qkv, b_qkv, W_out, b_out):
    bf16 = ml_dtypes.bfloat16
    x = np.asarray(x, dtype=np.float32).reshape(R, D)
    xT = np.ascontiguousarray(x.T)                     # [D, R]
    xk = np.ascontiguousarray(
        xT.reshape(KT8, 128, R).transpose(1, 0, 2))    # [128, KT8, R]
    x_hi, x_lo = _split8(xk)
    W_qkv = np.asarray(W_qkv, dtype=np.float32)
    W_out = np.asarray(W_out, dtype=np.float32)
    b_qkv = np.asarray(b_qkv, dtype=np.float32)
    mask = np.asarray(attention_mask).astype(np.float32)
    maskv = np.ascontiguousarray(
        mask.reshape(B, NKT, 128).transpose(2, 0, 1).reshape(128, B * NKT))
    maskcd = np.ascontiguousarray(maskv * CD)

    def _wsplit(w):  # [1024, 128] -> pair [128, KT8, 2, 128] + hi [128, KT8, 128]
        wk = np.ascontiguousarray(
            (w * SW).reshape(KT8, 128, 128).transpose(1, 0, 2))
        hi, lo = _split8(wk)
        pair = np.ascontiguousarray(
            np.stack([hi, lo], axis=2))                # [128, KT8, 2, 128]
        return pair, np.ascontiguousarray(hi)

    in_maps = []
    for c in range(CORES):
        s = slice(128 * c, 128 * (c + 1))
        wqp, wqh = _wsplit(W_qkv[:, s])
        wkp, wkh = _wsplit(W_qkv[:, D:][:, s])
        wvp, wvh = _wsplit(W_qkv[:, 2 * D:][:, s])
        in_maps.append({
            "xhi": x_hi, "xlo": x_lo,
            "wqp": wqp, "wkp": wkp, "wvp": wvp,
            "wqh": wqh, "wkh": wkh, "wvh": wvh,
            "wo": np.ascontiguousarray(W_out[s, :] * SW).astype(bf16),
            "bqkv": np.ascontiguousarray(
                np.stack([b_qkv[s], b_qkv[D:][s], b_qkv[2 * D:][s]]) * SW),
            "maskv": maskv,
            "maskcd": maskcd,
        })
    return in_maps


def timeline_estimate_ns(zero_bias=True):
    """Cost-model makespan of the per-core program (no HW needed)."""
    from concourse.timeline_sim import TimelineSim
    return TimelineSim(_get_nc(zero_bias), trace=False).simulate()


def run(trace=False, **inputs):
    zb = not np.any(np.asarray(inputs["b_qkv"]))
    nc = _get_nc(zb)
    b_out = np.asarray(inputs["b_out"], dtype=np.float32)
    in_maps = _make_in_maps(**inputs)
    try:
        res = run_bass_kernel_spmd(
            nc, in_maps, core_ids=list(range(CORES)), trace=trace,
        )
    except (ImportError, ModuleNotFoundError):
        res = run_bass_kernel_spmd(
            nc, in_maps, core_ids=list(range(CORES)), trace=False,
        )
    acc = np.zeros((D, R), dtype=np.float32)
    for r in res.results:
        acc += r["outT"].astype(np.float32)
    out = (np.ascontiguousarray(acc.T) / OUT_SCALE + b_out).reshape(B, N, D)
    return out, res


def kernel(**inputs):
    out, _ = run(trace=False, **inputs)
    return out


# revision 36
# speedup vs baseline: 1.1935x; 1.0089x over previous
"""Multi-head attention (B=2, N=2048, D=1024, H=16) on 8 trn2 NeuronCores.

Sharding: tensor-parallel over heads - core c owns heads (2c, 2c+1); host sums
the 8 full-size partial outputs (row-sharded W_out product) and transposes.

Device dataflow per core (fp8 DoubleRow matmuls where precision allows):
  phase 1 (QKV projections, 0.75 cyc/row): weights are host-split into
    (hi, lo) fp8e4 pairs; x is host-split into x_hi/x_lo fp8e4.
    Per k-tile: one DR matmul (w_hi,w_lo)x(x_hi bcast) + per k-tile-pair one
    DR matmul (w_hi,w_hi)x(x_lo pair) -> full-term fp32 PSUM (drops only the
    O(2^-8) lo*lo term, bf16-class accuracy).
    q -> fp8 (one-sided), k -> (hi,lo) fp8 pair, v -> bf16 natural [keys,dims]
    with a cd*mask denominator column appended.
  phase 2 (attention): scoresT = (k_hi,k_lo) DR-matmul (q,q) at 0.5 cyc/row
    (k at full precision, q one-sided fp8).  exp on ACT engine (bf16 out),
    a fraction of k-tiles on DVE via a custom 8-stage squaring op
    ((1+t/8+t^2/128)^8).  ctx natural orientation: out[queries,65] bf16
    matmuls filling all 128 output partitions (65 cyc/instr); denominators
    ride along as column 64 (cd*mask); gpsimd normalize_recip; DMA-transpose
    [128q, 2hx64d] -> ctxT.
  phase 3: outT_partial = wo^T ctxT (bf16), partials DMA'd as bf16; host sums.
"""

import os
import sys
import functools

import numpy as np
import ml_dtypes

for _p in (
    "/root/.axon_site",
    "/root/.axon_site/_ro/trn_rl_repo",
    "/root/.axon_site/_ro/pypackages",
    "/opt/trn_rl_repo",
):
    if os.path.isdir(_p) and _p not in sys.path:
        sys.path.append(_p)

def _ensure_axon():
    """Register the axon PJRT backend if sitecustomize didn't run."""
    import jax
    try:
        backends = jax._src.xla_bridge._backend_factories
        if "axon" in backends:
            return
    except Exception:
        pass
    try:
        from trn_agent_boot.trn_boot import boot
        os.environ.setdefault("AXON_POOL_SVC_OVERRIDE", "127.0.0.1")
        os.environ.setdefault("AXON_LOOPBACK_RELAY", "1")
        boot(os.environ["TRN_TERMINAL_PRECOMPUTED_JSON"],
             "/opt/axon/libaxon_pjrt.so")
    except Exception:
        pass


_ensure_axon()

import concourse.tile as tile
from concourse import bacc, mybir
from concourse.alu_op_type import AluOpType
from concourse.bass_utils import run_bass_kernel_spmd

BF16 = mybir.dt.bfloat16
F8 = mybir.dt.float8e4
F32 = mybir.dt.float32
DR = mybir.MatmulPerfMode.DoubleRow

B, N, D, H = 2, 2048, 1024, 16
DK = D // H            # 64
CORES = 8
HPC = H // CORES       # 2 heads per core
R = B * N              # 4096 rows total
RB = N                 # rows per batch
KT8 = D // 128         # 8 k-tiles over the model dim
QC = RB // 512         # 4 query chunks of 512 per batch
NKT = RB // 128        # 16 key tiles of 128 per batch
OT = D // 128          # 8 output-dim tiles

SW = 32.0              # weight pre-scale before fp8 split
CD = 1.0 / 8.0         # denominator column constant
EXP_SCALE = (DK ** -0.5) / (SW * SW)
OUT_SCALE = SW * SW / CD          # psum partial = OUT_SCALE * true partial
import os as _os
DVE_EXP_KTS = tuple(int(x) for x in _os.environ.get(
    "K_DVE_KTS", "2,6,10,14").split(",") if x != "")
QK_ENG = _os.environ.get("K_QK_ENG", "v")      # q/k psum->sbuf copy engine
CTXF_ENG = _os.environ.get("K_CTXF_ENG", "v")  # ctx psum->sbuf copy engine
P3_ENGS = _os.environ.get("K_P3_ENGS", "svsvsvsv")
V_ENG = _os.environ.get("K_V_ENG", "v")

# ---------------------------------------------------------------- custom exp
from concourse import dve_ops as _dve_ops
from concourse.dve_spec import AluOp, Bin, Spec, Src0, One, C0, lower as _dve_lower
from concourse.dve_spec import _has_src1
from concourse.dve_uop import DveOpSpec


def _exp_sq3_ref(in0, in1, s0, s1, imm2):
    d = in0.astype(np.float32) * np.float32(s0)
    e = d + np.float32(1.0)
    base = e * e + d * d
    r1 = base * base
    r2 = r1 * r1
    return r2 * r2


def _register_exp_op():
    name = "EXP_SQ3_ANT"
    if name in _dve_ops._SUB_OPCODE_FOR_NAME:
        return next(o for o in _dve_ops.OPS if o.name == name)
    d = Bin(AluOp.MULTIPLY, Src0, C0)
    e = Bin(AluOp.ADD, d, One)
    base = Bin(AluOp.ADD, Bin(AluOp.MULTIPLY, e, e), Bin(AluOp.MULTIPLY, d, d))
    r1 = Bin(AluOp.MULTIPLY, base, base)
    r2 = Bin(AluOp.MULTIPLY, r1, r1)
    r3 = Bin(AluOp.MULTIPLY, r2, r2)
    spec = Spec(body=r3, reference=_exp_sq3_ref)
    _dve_ops._SUB_OPCODE_FOR_NAME[name] = 17
    shas = {}
    for ver in ("v3", "v4"):
        dspec = DveOpSpec(name=name, opcode=17, uops=_dve_lower(spec, ver=ver),
                          rd1_en=_has_src1(spec))
        shas[ver] = dspec.sha(ver)
    op = _dve_ops.DveOp(name, spec, subdim=False, uops_sha=shas)
    _dve_ops.OPS.append(op)
    _dve_ops.CUSTOM_DVE_SPECS[name] = spec
    return op


EXP_OP = _register_exp_op()


# ------------------------------------------------------------------- program
def _build_nc(ZERO_BIAS=True):
    nc = bacc.Bacc(
        "TRN2", target_bir_lowering=False, debug=False, enable_asserts=False,
        num_devices=CORES,
    )

    xhi_d = nc.dram_tensor("xhi", [128, KT8, R], F8, kind="ExternalInput").ap()
    xlo_d = nc.dram_tensor("xlo", [128, KT8, R], F8, kind="ExternalInput").ap()
    # (hi, lo) weight pairs + hi-only tensors, per projection
    wqp_d = nc.dram_tensor("wqp", [128, KT8, 2, 128], F8, kind="ExternalInput").ap()
    wkp_d = nc.dram_tensor("wkp", [128, KT8, 2, 128], F8, kind="ExternalInput").ap()
    wvp_d = nc.dram_tensor("wvp", [128, KT8, 2, 128], F8, kind="ExternalInput").ap()
    wqh_d = nc.dram_tensor("wqh", [128, KT8, 128], F8, kind="ExternalInput").ap()
    wkh_d = nc.dram_tensor("wkh", [128, KT8, 128], F8, kind="ExternalInput").ap()
    wvh_d = nc.dram_tensor("wvh", [128, KT8, 128], F8, kind="ExternalInput").ap()
    wo_d = nc.dram_tensor("wo", [128, D], BF16, kind="ExternalInput").ap()
    bqkv_d = nc.dram_tensor("bqkv", [3, 128], F32, kind="ExternalInput").ap()
    maskv_d = nc.dram_tensor("maskv", [128, B * NKT], F32, kind="ExternalInput").ap()
    maskcd_d = nc.dram_tensor("maskcd", [128, B * NKT], F32, kind="ExternalInput").ap()
    outT_d = nc.dram_tensor("outT", [D, R], BF16, kind="ExternalOutput").ap()

    with tile.TileContext(nc) as tc:
        with (
            tc.tile_pool(name="persist", bufs=1) as persist,
            tc.tile_pool(name="xt", bufs=int(_os.environ.get("K_XTB", "3"))) as xt_pool,
            tc.tile_pool(name="aw", bufs=int(_os.environ.get("K_AWB", "2"))) as aw_pool,
            tc.tile_pool(name="small", bufs=int(_os.environ.get("K_SMB", "4"))) as small,
            tc.tile_pool(name="outsb", bufs=int(_os.environ.get("K_OSB", "2"))) as outsb_pool,
            tc.tile_pool(name="psmm", bufs=2, space="PSUM") as psmm_pool,
            tc.tile_pool(name="pssc", bufs=2, space="PSUM") as pssc_pool,
            tc.tile_pool(name="psctx", bufs=2, space="PSUM") as psctx_pool,
        ):
            # ---- weights / consts ----
            wqp = persist.tile([128, KT8, 2, 128], F8, tag="wqp")
            wkp = persist.tile([128, KT8, 2, 128], F8, tag="wkp")
            wvp = persist.tile([128, KT8, 2, 128], F8, tag="wvp")
            wqh = persist.tile([128, KT8, 128], F8, tag="wqh")
            wkh = persist.tile([128, KT8, 128], F8, tag="wkh")
            wvh = persist.tile([128, KT8, 128], F8, tag="wvh")
            wo_sb = persist.tile([128, D], BF16, tag="wo")
            bq_sb = persist.tile([128, 1], F32, tag="bq")
            bk_sb = persist.tile([128, 1], F32, tag="bk")
            bv_rep = persist.tile([128, 128], F32, tag="bv")
            maskv = persist.tile([128, B * NKT], F32, tag="maskv")
            maskcd = persist.tile([128, B * NKT], F32, tag="maskcd")

            def load_consts():
                # wqp/wqh already issued before the first x_lo DMA
                for w_sb, w_d in ((wkp, wkp_d), (wkh, wkh_d),
                                  (wvp, wvp_d), (wvh, wvh_d),
                                  (wo_sb, wo_d)):
                    nc.sync.dma_start(out=w_sb, in_=w_d)
                nc.sync.dma_start(
                    out=bq_sb, in_=bqkv_d[0:1, :].rearrange("o p -> p o"))
                nc.sync.dma_start(
                    out=bk_sb, in_=bqkv_d[1:2, :].rearrange("o p -> p o"))
                nc.sync.dma_start(
                    out=bv_rep, in_=bqkv_d[2:3, :].partition_broadcast(128))
                nc.sync.dma_start(out=maskv, in_=maskv_d)
                nc.sync.dma_start(out=maskcd, in_=maskcd_d)

            # ---- persistent activations ----
            qt = [persist.tile([128, RB], F8, tag=f"qt{b}", name=f"qt{b}")
                  for b in range(B)]
            kt2 = [persist.tile([128, 2, RB], F8, tag=f"kt{b}", name=f"kt{b}")
                   for b in range(B)]
            # V natural [keys, kt, h, 66]: cols 0-63 v, col 64 cd*mask
            v_sb = [persist.tile([128, NKT, HPC, 66], BF16, tag=f"v{b}",
                                 name=f"v{b}") for b in range(B)]
            ctxT = [persist.tile([128, RB], BF16, tag=f"ctxT{b}",
                                 name=f"ctxT{b}") for b in range(B)]

            def fill_cd_cols(b):
                # v ones-columns: cd * mask, broadcast over heads
                nc.gpsimd.tensor_copy(
                    out=v_sb[b][:, :, :, 64:65].squeeze(3),
                    in_=maskcd[:, b * NKT:(b + 1) * NKT]
                        .unsqueeze(2).broadcast_to([128, NKT, HPC]),
                )

            # ---- phase 1: QKV projections (fp8 DR, 3-term) ----
            def proj_matmuls(ps, xhi_t, xlo_t, w_pair, w_hi, n0, n1,
                             w_is_lhs):
                """12 DR matmuls accumulating x @ w into ps[128, n1-n0]."""
                first = True
                for kt in range(KT8):
                    if w_is_lhs:
                        nc.tensor.matmul(
                            ps, lhsT=w_pair[:, kt, :, :],
                            rhs=xhi_t[:, kt, n0:n1].unsqueeze(1)
                                .broadcast_to([128, 2, n1 - n0]),
                            start=first, stop=False, perf_mode=DR)
                    else:
                        nc.tensor.matmul(
                            ps, lhsT=xhi_t[:, kt, n0:n1].unsqueeze(1)
                                .broadcast_to([128, 2, n1 - n0]),
                            rhs=w_pair[:, kt, :, :],
                            start=first, stop=False, perf_mode=DR)
                    first = False
                for tp in range(KT8 // 2):
                    last = tp == KT8 // 2 - 1
                    if w_is_lhs:
                        nc.tensor.matmul(
                            ps, lhsT=w_hi[:, 2 * tp:2 * tp + 2, :],
                            rhs=xlo_t[:, 2 * tp:2 * tp + 2, n0:n1],
                            start=False, stop=last, perf_mode=DR)
                    else:
                        nc.tensor.matmul(
                            ps, lhsT=xlo_t[:, 2 * tp:2 * tp + 2, n0:n1],
                            rhs=w_hi[:, 2 * tp:2 * tp + 2, :],
                            start=False, stop=last, perf_mode=DR)

            def phase1_dma(b, chb, half=None):
                roff = b * RB + chb * 512
                if half in (None, 0):
                    xhi_t = xt_pool.tile([128, KT8, 512], F8, tag="xhi")
                    nc.sync.dma_start(out=xhi_t,
                                      in_=xhi_d[:, :, roff:roff + 512])
                    phase1_dma.hi = xhi_t
                    if half == 0:
                        return None
                xlo_t = xt_pool.tile([128, KT8, 512], F8, tag="xlo")
                nc.sync.dma_start(out=xlo_t, in_=xlo_d[:, :, roff:roff + 512])
                return phase1_dma.hi, xlo_t

            def phase1_chunk(b, chb, pre=None, defer_v=False):
                cols = slice(chb * 512, (chb + 1) * 512)
                xhi_t, xlo_t = pre if pre is not None else phase1_dma(b, chb)
                # Q
                qk_eng = nc.vector if QK_ENG == "v" else nc.scalar
                ps = psmm_pool.tile([128, 512], F32, tag="mm", name="qps")
                proj_matmuls(ps, xhi_t, xlo_t, wqp, wqh, 0, 512, True)
                qk_eng.tensor_scalar_add(
                    out=qt[b][:, cols], in0=ps, scalar1=bq_sb)
                # K -> hi/lo pair
                ps = psmm_pool.tile([128, 512], F32, tag="mm", name="kps")
                proj_matmuls(ps, xhi_t, xlo_t, wkp, wkh, 0, 512, True)
                qk_eng.tensor_scalar_add(
                    out=kt2[b][:, 0, cols], in0=ps, scalar1=bk_sb)
                qk_eng.scalar_tensor_tensor(
                    out=kt2[b][:, 1, cols], in0=ps, scalar=bk_sb,
                    in1=kt2[b][:, 0, cols], op0=AluOpType.add,
                    op1=AluOpType.subtract)
                # V (natural): out rows = x columns -> 4 sub-tiles of 128
                def emit_v():
                  for sub in range(4):
                    rt = chb * 4 + sub
                    n0 = sub * 128
                    vps = psmm_pool.tile([128, 128], F32, tag="mm",
                                         name=f"vps{b}{chb}{sub}")
                    proj_matmuls(vps, xhi_t, xlo_t, wvp, wvh, n0, n0 + 128,
                                 False)
                    mcol = maskv[:, b * NKT + rt:b * NKT + rt + 1]
                    for h in range(HPC):
                        vd = v_sb[b][:, rt, h, 0:64]
                        if ZERO_BIAS:
                            if V_ENG == "s":
                                nc.scalar.mul(
                                    out=vd, in_=vps[:, h * 64:(h + 1) * 64],
                                    mul=mcol)
                            else:
                                nc.vector.tensor_scalar_mul(
                                    out=vd, in0=vps[:, h * 64:(h + 1) * 64],
                                    scalar1=mcol)
                        else:
                            nc.vector.tensor_add(
                                out=vd, in0=vps[:, h * 64:(h + 1) * 64],
                                in1=bv_rep[:, h * 64:(h + 1) * 64])
                            nc.gpsimd.tensor_scalar_mul(
                                out=vd, in0=vd, scalar1=mcol)
                if defer_v:
                    return emit_v
                emit_v()
                return None

            # ---- phase 2: attention ----
            def attn_alloc():
                # ctx accumulators: [128, 4, 128] x2 (h), slot j = qsub
                return [psctx_pool.tile([128, 4, 128], F32, tag="ctx",
                                        name=f"ctxps{h}") for h in range(HPC)]

            def attn_ctx(b, cps, awt, kt):
                # one start per PSUM tile: start zeroing covers the whole
                # tile, so only (kt0, j0) starts; other regions accumulate
                # onto the zeroed tile
                for h in range(HPC):
                    for j in range(4):
                        nc.tensor.matmul(
                            cps[h][:, j, 0:65],
                            lhsT=awt[:, kt, h, j * 128:(j + 1) * 128],
                            rhs=v_sb[b][:, kt, h, 0:65],
                            start=(kt == 0 and j == 0),
                            stop=(kt == NKT - 1),
                            skip_group_check=True)

            def attn_kts(b, qc, cps, kts, pend):
                qs = qc * 512
                for kt in kts:
                    ks = kt * 128
                    sc = pssc_pool.tile([128, HPC, 512], F32, tag="sc",
                                        name="scps")
                    for h in range(HPC):
                        p0 = h * 64
                        nc.tensor.matmul(
                            sc[:, h, :],
                            lhsT=kt2[b][p0:p0 + 64, :, ks:ks + 128],
                            rhs=qt[b][p0:p0 + 64, qs:qs + 512].unsqueeze(1)
                                .broadcast_to([64, 2, 512]),
                            start=True, stop=True, perf_mode=DR,
                            tile_position=(p0, 0))
                    awt = aw_pool.tile([128, NKT, HPC, 512], BF16, tag="aw",
                                       name="awt") if kt == 0 else attn_kts.awt
                    attn_kts.awt = awt
                    if kt in DVE_EXP_KTS:
                        nc.vector._custom_dve(
                            EXP_OP, out=awt[:, kt, :, :], in0=sc,
                            s0=EXP_SCALE / 16.0)
                    else:
                        nc.scalar.activation(
                            out=awt[:, kt, :, :], in_=sc,
                            func=mybir.ActivationFunctionType.Exp,
                            scale=EXP_SCALE)
                    # software pipeline: run the PREVIOUS kt's ctx now, so the
                    # PE never blocks on this kt's exp round-trip
                    while len(pend) >= int(_os.environ.get('K_PEND', '6')):
                        attn_ctx(b, cps, awt, pend.pop(0))
                    pend.append(kt)

            def attn_finish(b, qc, cps):
                rcp = small.tile([128, HPC, 4], F32, tag="rcp", name="rcp")
                ctxn = small.tile([128, 4, HPC, 64], BF16, tag="ctxn",
                                  name="ctxn")
                for h in range(HPC):
                    nc.vector.reciprocal(
                        rcp[:, h, :], cps[h][:, :, 64:65].squeeze(2))
                    nc.vector.tensor_mul(
                        out=ctxn[:, :, h, :], in0=cps[h][:, :, 0:64],
                        in1=rcp[:, h, :].unsqueeze(2).broadcast_to(
                            [128, 4, 64]))
                for j in range(4):
                    qs = b * 0 + qc * 512 + j * 128
                    nc.sync.dma_start_transpose(
                        out=ctxT[b][:, qs:qs + 128],
                        in_=ctxn[:, j, :, :])

            # ---- phase 3 ----
            # copy engines: nc.vector / nc.gpsimd / nc.scalar mix per ot
            p3_osb = {}

            def phase3_half(b, rc, half, engs=None):
                engs = engs or (P3_ENGS[:4] if half == 0 else P3_ENGS[4:])
                emap = {"v": nc.vector, "g": nc.gpsimd, "s": nc.scalar}
                if half == 0:
                    p3_osb[(b, rc)] = outsb_pool.tile(
                        [128, OT, 512], BF16, tag="osb", name=f"osb{b}{rc}")
                osb = p3_osb[(b, rc)]
                for i in range(4):
                    ot = half * 4 + i
                    ps = psmm_pool.tile([128, 512], F32, tag="mm",
                                        name="outps")
                    nc.tensor.matmul(
                        ps, lhsT=wo_sb[:, ot * 128:(ot + 1) * 128],
                        rhs=ctxT[b][:, rc * 512:(rc + 1) * 512],
                        start=True, stop=True)
                    eng = emap[engs[i]]
                    if eng is nc.scalar:
                        eng.copy(out=osb[:, ot, :], in_=ps)
                    else:
                        eng.tensor_copy(out=osb[:, ot, :], in_=ps)
                if half == 1:
                    cs = b * RB + rc * 512
                    outT_r = outT_d.rearrange("(t p) r -> p t r", p=128)
                    nc.sync.dma_start(out=outT_r[:, :, cs:cs + 512], in_=osb)

            def phase3(b, rc, engs=None):
                engs = engs or P3_ENGS
                phase3_half(b, rc, 0, engs[:4])
                phase3_half(b, rc, 1, engs[4:])

            # ---- schedule ----
            def attn(b, qc, hooks=None):
                cps = attn_alloc()
                pend = []
                lo = 0
                for at in sorted(hooks or {}):
                    attn_kts(b, qc, cps, list(range(lo, at)), pend)
                    hooks[at]()
                    lo = at
                attn_kts(b, qc, cps, list(range(lo, NKT)), pend)
                while pend:
                    attn_ctx(b, cps, attn_kts.awt, pend.pop(0))
                attn_finish(b, qc, cps)

            phase1_dma(0, 0, half=0)
            nc.sync.dma_start(out=wqp, in_=wqp_d)
            nc.sync.dma_start(out=wqh, in_=wqh_d)
            pre00 = phase1_dma(0, 0, half=1)
            load_consts()
            fill_cd_cols(0)
            fill_cd_cols(1)
            v00 = phase1_chunk(0, 0, pre=pre00, defer_v=True)
            attn(0, 0, {1: v00,
                        4: lambda: phase1_chunk(0, 1),
                        8: lambda: phase1_chunk(0, 2),
                        12: lambda: phase1_chunk(0, 3)})
            attn(0, 1, {4: lambda: phase1_chunk(1, 0),
                        12: lambda: phase1_chunk(1, 1)})
            attn(0, 2, {4: lambda: phase1_chunk(1, 2),
                        12: lambda: phase3_half(0, 0, 0)})
            attn(0, 3, {4: lambda: phase3_half(0, 0, 1),
                        8: lambda: phase1_chunk(1, 3),
                        12: lambda: phase3_half(0, 1, 0)})
            attn(1, 0, {4: lambda: phase3_half(0, 1, 1),
                        12: lambda: phase3_half(0, 2, 0)})
            attn(1, 1, {4: lambda: phase3_half(0, 2, 1),
                        12: lambda: phase3_half(0, 3, 0)})
            attn(1, 2, {4: lambda: phase3_half(0, 3, 1),
                        12: lambda: phase3_half(1, 0, 0)})
            attn(1, 3, {4: lambda: phase3_half(1, 0, 1),
                        8: lambda: phase3_half(1, 1, 0),
                        12: lambda: phase3_half(1, 1, 1)})
            phase3(1, 2, engs="svsvsvsv")
            phase3(1, 3, engs="svsvsvsv")

    nc.compile()
    return nc


@functools.lru_cache(maxsize=2)
def _get_nc(zero_bias=True):
    return _build_nc(zero_bias)


def _split8(a):
    f8 = ml_dtypes.float8_e4m3
    hi = a.astype(f8)
    lo = (a - hi.astype(np.float32)).astype(f8)
    return hi, lo


def _make_in_maps(x, attention_mask, W_qkv, b_qkv, W_out, b_out):
    bf16 = ml_dtypes.bfloat16
    x = np.asarray(x, dtype=np.float32).reshape(R, D)
    xT = np.ascontiguousarray(x.T)                     # [D, R]
    xk = np.ascontiguousarray(
        xT.reshape(KT8, 128, R).transpose(1, 0, 2))    # [128, KT8, R]
    x_hi, x_lo = _split8(xk)
    W_qkv = np.asarray(W_qkv, dtype=np.float32)
    W_out = np.asarray(W_out, dtype=np.float32)
    b_qkv = np.asarray(b_qkv, dtype=np.float32)
    mask = np.asarray(attention_mask).astype(np.float32)
    maskv = np.ascontiguousarray(
        mask.reshape(B, NKT, 128).transpose(2, 0, 1).reshape(128, B * NKT))
    maskcd = np.ascontiguousarray(maskv * CD)

    def _wsplit(w):  # [1024, 128] -> pair [128, KT8, 2, 128] + hi [128, KT8, 128]
        wk = np.ascontiguousarray(
            (w * SW).reshape(KT8, 128, 128).transpose(1, 0, 2))
        hi, lo = _split8(wk)
        pair = np.ascontiguousarray(
            np.stack([hi, lo], axis=2))                # [128, KT8, 2, 128]
        return pair, np.ascontiguousarray(hi)

    in_maps = []
    for c in range(CORES):
        s = slice(128 * c, 128 * (c + 1))
        wqp, wqh = _wsplit(W_qkv[:, s])
        wkp, wkh = _wsplit(W_qkv[:, D:][:, s])
        wvp, wvh = _wsplit(W_qkv[:, 2 * D:][:, s])
        in_maps.append({
            "xhi": x_hi, "xlo": x_lo,
            "wqp": wqp, "wkp": wkp, "wvp": wvp,
            "wqh": wqh, "wkh": wkh, "wvh": wvh,
            "wo": np.ascontiguousarray(W_out[s, :] * SW).astype(bf16),
            "bqkv": np.ascontiguousarray(
                np.stack([b_qkv[s], b_qkv[D:][s], b_qkv[2 * D:][s]]) * SW),
            "maskv": maskv,
            "maskcd": maskcd,
        })
    return in_maps


def timeline_estimate_ns(zero_bias=True):
    """Cost-model makespan of the per-core program (no HW needed)."""
    from concourse.timeline_sim import TimelineSim
    return TimelineSim(_get_nc(zero_bias), trace=False).simulate()


def run(trace=False, **inputs):
    zb = not np.any(np.asarray(inputs["b_qkv"]))
    nc = _get_nc(zb)
    b_out = np.asarray(inputs["b_out"], dtype=np.float32)
    in_maps = _make_in_maps(**inputs)
    try:
        res = run_bass_kernel_spmd(
            nc, in_maps, core_ids=list(range(CORES)), trace=trace,
        )
    except (ImportError, ModuleNotFoundError):
        res = run_bass_kernel_spmd(
            nc, in_maps, core_ids=list(range(CORES)), trace=False,
        )
    acc = np.zeros((D, R), dtype=np.float32)
    for r in res.results:
        acc += r["outT"].astype(np.float32)
    out = (np.ascontiguousarray(acc.T) / OUT_SCALE + b_out).reshape(B, N, D)
    return out, res


def kernel(**inputs):
    out, _ = run(trace=False, **inputs)
    return out
